# revision 4
# baseline (speedup 1.0000x reference)
"""Causal self-attention (B=4, T=2048, E=1024, H=16) on 8 trn2 NeuronCores.

Sharding: core c -> (batch b = c // 2, head-group hg = c % 2); each core owns
one batch element and 8 of the 16 heads (data parallel on B, tensor parallel
on heads).  No cross-core communication.

v3 design (compensated-fp8 projection + exp split across ScalarE/DVE):
  - QKV projection in fp8e4m3 DoubleRow perf mode: x and w are split hi/lo
    (w pre-scaled x32 on host; lo = fp8(residual), unscaled so both terms
    accumulate in one psum group).  psum = x8h*w8h + x8h*w8l + x8l*w8h over
    4 e-tile pairs = 12 DR matmuls per 128-col group (2.66x fewer PE cycles
    than the f32r version at ~0.12% element error).  q/k bias (x32) added on
    DVE during the psum->sbuf bf16 copy; v bias via a ones-row matmul.
  - QK: bf16 row-tiled head pairs (2 heads per pass), causal trim per
    j-tile; scores land in psum as 1024*score (both q,k carry x32).
  - exp: scale 0.125/1024 on ScalarE; a fraction of the full (non-diagonal)
    j-tiles instead run a 2-instruction custom-DVE exp (deg-4 Horner 16th
    root + 4 squarings, validated at the bf16-output error floor) to
    offload the ScalarE bottleneck.  Causal mask via gpsimd affine_select
    on the diagonal band only (diagonal tiles always take the ScalarE path).
  - PV flipped: pt stationary, v moving ([j, d+ones] 65 cols).  The ones
    column is 32.0 so the denominator matches v's x32 scale and the
    normalization (reciprocal + per-partition-scalar multiply) cancels it.
  - Output normalization multiplies run on gpsimd (Pool) to unload DVE.
  - Output written as y[t, c]; host concatenates without transposing.
"""

import sys

sys.path.insert(0, "/opt/trn_rl_repo")

import numpy as np

N_CORES = 8
B, T, E = 4, 2048, 1024
H, D = 16, 64
C = E                 # q/k/v channel count (4th qkv chunk unused)
HPC = H // 2          # heads per core
CC = HPC * D          # per-core channels = 512
EP = 4                # e-tile pairs (contraction 1024 = 4 pairs x 256)
TB = T // 512         # 4 t/i blocks of 512
NJ = T // 128         # 16 j-tiles of 128
PAIRS = HPC // 2      # 4 head pairs per core

WSCALE = 32.0         # host pre-scale on w/b so fp8 hi/lo are well-ranged

_cache = {}

# ---- custom DVE exp ops ---------------------------------------------------

_EXP_OPS = {}


def _register_exp_ops():
    if _EXP_OPS:
        return _EXP_OPS["p4"], _EXP_OPS["sq4"]
    import re

    import concourse.dve_ops as dops
    from concourse.dve_ops import DveOp
    from concourse.dve_spec import C0, C1, C2, One, Spec, Src0, Src1, sq

    s = Src0
    body_p4 = ((((Src1 * s + C0) * s + C1) * s + C2) * s + One)

    def _ref_p4(in0, in1, s0, s1, imm2):
        r = ((((in1 * in0 + s0) * in0 + s1) * in0 + imm2) * in0 + 1.0)
        return r.astype(np.float32)

    x2 = sq(Src0)
    x4 = sq(x2)
    x8 = sq(x4)
    body_sq4 = sq(x8)

    def _ref_sq4(in0, in1, s0, s1, imm2):
        return ((((in0 * in0) ** 2) ** 2) ** 2).astype(np.float32)

    def make(name, body, ref, row):
        dops._SUB_OPCODE_FOR_NAME[name] = row
        op = DveOp(name, Spec(body=body, reference=ref), subdim=False,
                   uops_sha={"v3": "?"})
        try:
            op.compile("v3")
        except ValueError as e:
            m = re.search(r"v3: ([0-9a-f]+) ", str(e))
            assert m, f"cannot parse uops sha from: {e}"
            dops._COMPILE_CACHE.pop((name, "v3"), None)
            object.__setattr__(op, "uops_sha", {"v3": m.group(1)})
        op.compile("v3")
        dops.OPS.append(op)
        dops.CUSTOM_DVE_SPECS[name] = op.spec
        return op

    row = max(dops._SUB_OPCODE_FOR_NAME.values())
    assert row + 2 < 0x20, "custom-DVE row field overflow"
    p4 = make("EXP_P4_ANT", body_p4, _ref_p4, row + 1)
    sq4 = make("EXP_SQ4_ANT", body_sq4, _ref_sq4, row + 2)
    _EXP_OPS.update(p4=p4, sq4=sq4)
    return p4, sq4


def _exp_coeffs():
    """P(u) = (((a u + b) u + c) u + d) u + 1 with P(K*st)^16 ~ exp(K16*st),
    K16 = 0.125/1024 (st holds 1024*score, exp wants 0.125*score).  Fit the
    16th root e^(u/16) by Chebyshev interpolation over u in [-6.5, 6.5]
    (covers |score| <= 6.5), normalize constant term to 1 (the resulting
    global e16 factor cancels in the softmax normalization)."""
    u0, n = 6.5, 5
    xk = np.cos(np.pi * (2 * np.arange(n) + 1) / (2 * n)) * u0
    V = np.vander(xk, n, increasing=True)
    coef = np.linalg.solve(V, np.exp(xk / 16.0))
    K = 0.125 / 1024.0
    coef = coef * (K ** np.arange(n))
    c0, c1, c2, c3, c4 = [float(v) for v in coef]
    return c4 / c0, c3 / c0, c2 / c0, c1 / c0


def _build_nc():
    import concourse.mybir as mybir
    import concourse.tile as tile
    from concourse import bacc

    p4_op, sq4_op = _register_exp_ops()
    CA, CB, CCo, CD = _exp_coeffs()

    f32 = mybir.dt.float32
    f32r = mybir.dt.float32r
    bf16 = mybir.dt.bfloat16
    fp8 = mybir.dt.float8e4
    Act = mybir.ActivationFunctionType
    DR = mybir.MatmulPerfMode.DoubleRow
    is_ge = mybir.AluOpType.is_ge

    nc = bacc.Bacc("TRN2", target_bir_lowering=False, debug=False)

    xT8h = nc.dram_tensor("xT8h", [E, T], fp8, kind="ExternalInput").ap()
    xT8l = nc.dram_tensor("xT8l", [E, T], fp8, kind="ExternalInput").ap()
    wqk8h = nc.dram_tensor("wqk8h", [E, 2 * CC], fp8, kind="ExternalInput").ap()
    wqk8l = nc.dram_tensor("wqk8l", [E, 2 * CC], fp8, kind="ExternalInput").ap()
    wv8h = nc.dram_tensor("wv8h", [E, CC], fp8, kind="ExternalInput").ap()
    wv8l = nc.dram_tensor("wv8l", [E, CC], fp8, kind="ExternalInput").ap()
    b_qk = nc.dram_tensor("b_qk", [128, 8], f32, kind="ExternalInput").ap()
    b_v = nc.dram_tensor("b_v", [1, CC], f32r, kind="ExternalInput").ap()
    ones_d = nc.dram_tensor("ones_d", [1, 128], f32r, kind="ExternalInput").ap()
    # flat [(I, it, p), (head, d)] == [T, CC] row-major; out DMAs use strided
    # APs so (head, d) runs stay 512B-contiguous.
    y_out = nc.dram_tensor("y_out", [T, CC], f32, kind="ExternalOutput").ap()

    def pair_rows(dram, k, csl=None):
        """[E, N] dram rows 256k..256k+255 -> [128, 2, n] AP (e-pair dim1)."""
        sl = dram[256 * k : 256 * (k + 1), :] if csl is None else \
            dram[256 * k : 256 * (k + 1), csl]
        return sl.rearrange("(two p) n -> p two n", two=2)

    with tile.TileContext(nc) as tc:
        with (
            tc.tile_pool(name="persist", bufs=1) as pp,
            tc.tile_pool(name="psum", bufs=1, space="PSUM") as psp,
            tc.tile_pool(name="xpool", bufs=2) as xp,
            tc.tile_pool(name="ptpool", bufs=12) as ptp,
            tc.tile_pool(name="upool", bufs=3) as up,
            tc.tile_pool(name="opool", bufs=1) as op,
        ):
            # ---- persistent SBUF state ----
            qk_sb = [pp.tile([128, T], bf16, name=f"qk{ct}") for ct in range(8)]
            # v plus a 32.0 column per head: [t-part, head, j-tile, 65]
            v1_sb = pp.tile([128, HPC, NJ, D + 1], bf16, name="v1")
            bqk_sb = pp.tile([128, 8], f32, name="bqk")
            bv_sb = pp.tile([1, CC], f32r, name="bv")
            ones_sb = pp.tile([1, 128], f32r, name="ones")
            acoef_sb = pp.tile([128, 1024], f32, name="acoef")
            wqh_t = []
            wql_t = []
            wvh_t = []
            wvl_t = []

            # softmax-denominator column: 32.0 matches v's x32 scale so the
            # per-row normalization cancels it exactly
            nc.gpsimd.memset(v1_sb[:, :, :, D : D + 1], WSCALE)
            nc.gpsimd.memset(acoef_sb, CA)

            xs_tb = {}

            def load_x(tb):
                tsl = slice(tb * 512, (tb + 1) * 512)
                xs = []
                for k in range(EP):
                    xh = xp.tile([128, 2, 512], fp8, tag=f"xh{k}", bufs=2,
                                 name=f"xh{k}_{tb}")
                    nc.sync.dma_start(out=xh, in_=pair_rows(xT8h, k, tsl))
                    xl = xp.tile([128, 2, 512], fp8, tag=f"xl{k}", bufs=2,
                                 name=f"xl{k}_{tb}")
                    nc.sync.dma_start(out=xl, in_=pair_rows(xT8l, k, tsl))
                    xs.append((xh, xl))
                xs_tb[tb] = xs

            # small constants, then x0/wqk interleaved per e-pair (the
            # exp-critical path: pair 0's q/k projection), then wv
            nc.sync.dma_start(out=bqk_sb, in_=b_qk)
            nc.sync.dma_start(out=bv_sb, in_=b_v)
            nc.sync.dma_start(out=ones_sb, in_=ones_d)
            tsl0 = slice(0, 512)
            xs0 = []
            # (host packs w_qk cols pr-major: pr*256+[0:128]=q, +[128:256]=k)
            for k in range(EP):
                xh = xp.tile([128, 2, 512], fp8, tag=f"xh{k}", bufs=2,
                             name=f"xh{k}_0")
                nc.sync.dma_start(out=xh, in_=pair_rows(xT8h, k, tsl0))
                xl = xp.tile([128, 2, 512], fp8, tag=f"xl{k}", bufs=2,
                             name=f"xl{k}_0")
                nc.sync.dma_start(out=xl, in_=pair_rows(xT8l, k, tsl0))
                xs0.append((xh, xl))
                # cols 0:256 = pair-0's q and k — the exp-critical path
                wqh = pp.tile([128, 2, 2 * CC], fp8, name=f"wqh{k}")
                nc.sync.dma_start(out=wqh[:, :, 0:256],
                                  in_=pair_rows(wqk8h, k, slice(0, 256)))
                wql = pp.tile([128, 2, 2 * CC], fp8, name=f"wql{k}")
                nc.sync.dma_start(out=wql[:, :, 0:256],
                                  in_=pair_rows(wqk8l, k, slice(0, 256)))
                wqh_t.append(wqh)
                wql_t.append(wql)
            for k in range(EP):
                nc.sync.dma_start(out=wqh_t[k][:, :, 256:512],
                                  in_=pair_rows(wqk8h, k, slice(256, 512)))
                nc.sync.dma_start(out=wql_t[k][:, :, 256:512],
                                  in_=pair_rows(wqk8l, k, slice(256, 512)))
            xs_tb[0] = xs0
            for k in range(EP):
                wvh = pp.tile([128, 2, CC], fp8, name=f"wvh{k}")
                nc.sync.dma_start(out=wvh, in_=pair_rows(wv8h, k))
                wvl = pp.tile([128, 2, CC], fp8, name=f"wvl{k}")
                nc.sync.dma_start(out=wvl, in_=pair_rows(wv8l, k))
                wvh_t.append(wvh)
                wvl_t.append(wvl)
            for k in range(EP):
                nc.sync.dma_start(out=wqh_t[k][:, :, 512:1024],
                                  in_=pair_rows(wqk8h, k, slice(512, 1024)))
                nc.sync.dma_start(out=wql_t[k][:, :, 512:1024],
                                  in_=pair_rows(wqk8l, k, slice(512, 1024)))

            def qk_terms(ct):
                co = (ct % 4) * 256 + (128 if ct >= 4 else 0)
                csl = slice(co, co + 128)
                terms = []
                for k in range(EP):
                    xh, xl = None, None  # bound at emit time via xs_tb
                    terms.append((k, "hh", csl))
                    terms.append((k, "hl", csl))
                    terms.append((k, "lh", csl))
                return terms

            def qkv_group_qk(tb, ct, lo=0, hi=12, cell=None):
                """Emit DR-term chunk [lo, hi) of the ct projection group;
                the last chunk appends the DVE bias-add."""
                tsl = slice(tb * 512, (tb + 1) * 512)
                xs = xs_tb[tb]
                terms = qk_terms(ct)
                if cell is None:
                    cell = {}
                if lo == 0:
                    cell["ps"] = psp.tile([128, 512], f32, tag="qp", bufs=2,
                                          name=f"psqk{ct}_{tb}")
                ps = cell["ps"]
                for i in range(lo, hi):
                    k, kind, csl = terms[i]
                    xh, xl = xs[k]
                    w = (wqh_t if kind[0] == "h" else wql_t)[k][:, :, csl]
                    x = xh if kind[1] == "h" else xl
                    nc.tensor.matmul(
                        ps, w, x,
                        start=(i == 0),
                        stop=(i == 11),
                        perf_mode=DR,
                        skip_group_check=True,
                    )
                if hi == 12:
                    # bias add on DVE (psum f32 + [128,1] bias -> sbuf bf16)
                    nc.vector.tensor_scalar_add(
                        qk_sb[ct][:, tsl], ps, bqk_sb[:, ct : ct + 1])

            def qkv_group_v(tb, k4, lo=0, hi=12, cell=None):
                xs = xs_tb[tb]
                tt = tb * 4 + k4
                csl = slice(k4 * 128, (k4 + 1) * 128)
                if cell is None:
                    cell = {}
                if lo == 0:
                    cell["ps"] = psp.tile([128, 512], f32, tag="qp", bufs=2,
                                          name=f"psv{tt}")
                    nc.tensor.matmul(
                        cell["ps"], ones_sb, bv_sb,
                        start=True, stop=False, skip_group_check=True,
                    )
                psv = cell["ps"]
                terms = [(k, kind) for k in range(EP)
                         for kind in ("hh", "hl", "lh")]
                for i in range(lo, hi):
                    k, kind = terms[i]
                    xh, xl = xs[k]
                    x = (xh if kind[1] == "h" else xl)[:, :, csl]
                    w = (wvh_t if kind[0] == "h" else wvl_t)[k]
                    nc.tensor.matmul(
                        psv, x, w,
                        start=False,
                        stop=(i == 11),
                        perf_mode=DR,
                        skip_group_check=True,
                    )
                if hi == 12:
                    nc.vector.tensor_copy(
                        v1_sb[:, :, tt, 0:D],
                        psv.rearrange("p (h d) -> p h d", d=D),
                    )

            def qk_chunks(tb, cts):
                steps = []
                for ct in cts:
                    cell = {}
                    for lo, hi in ((0, 4), (4, 8), (8, 12)):
                        steps.append(
                            lambda c=ct, l=lo, h=hi, ce=cell:
                            qkv_group_qk(tb, c, l, h, ce))
                return steps

            def v_chunks(tb):
                steps = []
                for g in range(4):
                    cell = {}
                    for lo, hi in ((0, 4), (4, 8), (8, 12)):
                        steps.append(
                            lambda k=g, l=lo, h=hi, ce=cell:
                            qkv_group_v(tb, k, l, h, ce))
                return steps

            def attn_block(I, nxt=(), pre=None):
                nj = 4 * I + 4  # causal j-tiles for this i-block
                yts = {}
                pts = {}
                ndve = {}

                def qk_exp(pr, J):
                    r = J - 4 * I
                    ws = 128 * r if r > 0 else 0  # causal trim offset
                    qt = qk_sb[pr]
                    kt = qk_sb[4 + pr]
                    jsl = slice(J * 128, (J + 1) * 128)
                    iwl = slice(I * 512 + ws, (I + 1) * 512)
                    st = psp.tile([128, 2, 512], f32, tag="st", bufs=2,
                                  name=f"st{pr}_{I}_{J}")
                    # QK row-tile pair: head A rows 0-63, head B 64-127
                    nc.tensor.matmul(
                        st[:, 0, ws:], kt[0:64, jsl], qt[0:64, iwl],
                        tile_position=(0, 0),
                    )
                    nc.tensor.matmul(
                        st[:, 1, ws:], kt[64:128, jsl], qt[64:128, iwl],
                        tile_position=(64, 0),
                    )
                    pt = ptp.tile([128, 2, 512], bf16, tag="pt",
                                  name=f"pt{pr}_{I}_{J}")
                    # route a fraction of full tiles through the DVE exp to
                    # unload ScalarE; only in late blocks, where ScalarE is
                    # saturated and DVE is idle (early blocks are PE-paced:
                    # ScalarE starves anyway, and DVE is busy with the
                    # projection copies).  Diagonal tiles (r >= 0, need
                    # masking/trim) always use ScalarE.
                    mod = {2: 3, 3: 2}.get(I, 0)
                    use_dve = (r < 0) and mod and (ndve["n"] % mod == 1)
                    if r < 0:
                        ndve["n"] += 1
                    if use_dve:
                        u = up.tile([128, 1024], f32, tag="u",
                                    name=f"u{pr}_{I}_{J}")
                        nc.vector._custom_dve(
                            p4_op, out=u,
                            in0=st.rearrange("p h w -> p (h w)"),
                            in1=acoef_sb, s0=CB, s1=CCo, imm2=CD)
                        nc.vector._custom_dve(
                            sq4_op, out=pt.rearrange("p h w -> p (h w)"),
                            in0=u)
                    else:
                        nc.scalar.activation(pt[:, :, ws:], st[:, :, ws:],
                                             Act.Exp, scale=0.125 / 1024.0)
                    if r >= 0:
                        # causal mask on the 128-wide diagonal band only:
                        # keep where i_band >= j (within-tile coords)
                        nc.gpsimd.affine_select(
                            out=pt[:, :, 128 * r : 128 * (r + 1)],
                            in_=pt[:, :, 128 * r : 128 * (r + 1)],
                            compare_op=is_ge,
                            fill=0.0,
                            base=0,
                            pattern=[[0, 2], [1, 128]],
                            channel_multiplier=-1,
                        )
                    pts[(pr, J)] = pt

                def pv(pr, J):
                    pt = pts.pop((pr, J))
                    ytA, ytB = yts[pr]
                    r = J - 4 * I
                    for h, yt in ((0, ytA), (1, ytB)):
                        for it in range(4):
                            if r > it:
                                continue  # i-tile fully masked for this j
                            # PSUM zeroing is bank-granular: only the FIRST
                            # region of each bank sets start=True
                            nc.tensor.matmul(
                                yt[:, it, 0 : D + 1],
                                pt[:, h, it * 128 : (it + 1) * 128],
                                v1_sb[:, 2 * pr + h, J, :],
                                start=(J == 0 and it == 0),
                                stop=(J == 4 * I + it),
                                skip_group_check=True,
                            )

                def out_stage(pr):
                    ytA, ytB = yts.pop(pr)
                    rec = op.tile([128, 2, 4], f32, tag="rec", bufs=2,
                                  name=f"rec{pr}_{I}")
                    yc = op.tile([128, 2, 4, D + 1], f32, tag="yc", bufs=2,
                                 name=f"yc{pr}_{I}")
                    ys = op.tile([128, 4, 2, D], f32, tag="ys", bufs=2,
                                 name=f"ys{pr}_{I}")
                    # copy psum->sbuf first: releases the yt banks earlier
                    nc.vector.tensor_copy(yc[:, 0, :, :], ytA[:, :, 0 : D + 1])
                    nc.vector.tensor_copy(yc[:, 1, :, :], ytB[:, :, 0 : D + 1])
                    # denominators live in column 64 of each (h, it) slot
                    nc.vector.reciprocal(rec[:, 0, :], yc[:, 0, :, D])
                    nc.vector.reciprocal(rec[:, 1, :], yc[:, 1, :, D])
                    for h in (0, 1):
                        for it in range(4):
                            # per-partition-scalar multiply on gpsimd (Pool)
                            nc.gpsimd.tensor_scalar_mul(
                                ys[:, it, h, :], yc[:, h, it, 0:D],
                                rec[:, h, it : it + 1])
                    # ys [i-part, it, h, d] -> y_out[I, it, p, 2pr+h, d]
                    import concourse.bass as bass

                    out_ap = bass.AP(
                        tensor=y_out.tensor,
                        offset=I * (4 * 128 * CC) + 2 * pr * D,
                        ap=[[CC, 128], [128 * CC, 4], [1, 2 * D]],
                    )
                    nc.sync.dma_start(
                        out=out_ap,
                        in_=ys.rearrange("p i h d -> p i (h d)"),
                    )

                def alloc_yt(pr):
                    # one full 2KB PSUM bank per head so the bank-granular
                    # start=True zeroing touches no other tile
                    yts[pr] = (
                        psp.tile([128, 4, 128], f32, tag="ytA", bufs=1,
                                 name=f"ytA{pr}_{I}"),
                        psp.tile([128, 4, 128], f32, tag="ytB", bufs=1,
                                 name=f"ytB{pr}_{I}"),
                    )

                ndve["n"] = I  # stagger DVE-routed tiles across blocks
                items = [(pr, J) for pr in range(PAIRS) for J in range(nj)]
                nxt = list(nxt)
                nsteps = len(nxt)
                popped = 0
                emitted = 0

                def emit_qk(k):
                    pr, J = items[k]
                    if J == 0:
                        if pre:
                            for fn in pre.get(pr, ()):
                                fn()
                        alloc_yt(pr)
                    qk_exp(pr, J)

                for k in range(len(items)):
                    while emitted < min(k + 4, len(items)):
                        emit_qk(emitted)
                        emitted += 1
                    pr, J = items[k]
                    pv(pr, J)
                    if J == nj - 1:
                        out_stage(pr)
                    # weave next t-block's QKV in small chunks so a long
                    # projection burst never delays the next QK
                    want = (k + 1) * nsteps // len(items)
                    while popped < want:
                        nxt[popped]()
                        popped += 1
                for fn in nxt[popped:]:
                    fn()

            # schedule: per-pair staging for EVERY block.  Block I weaves
            # only what block I+1 needs at its start (v groups + pair 0's
            # q/k); pairs 1-3's q/k groups emit as `pre` inside block I+1.
            qkv_group_qk(0, 0)
            qkv_group_qk(0, 4)
            for g in range(4):
                qkv_group_v(0, g)
            pres = {
                0: {
                    pr: qk_chunks(0, [pr, 4 + pr])
                    for pr in range(1, PAIRS)
                }
            }
            for I in range(TB):
                nxt = []
                if I + 1 < TB:
                    load_x(I + 1)
                    nxt = v_chunks(I + 1) + qk_chunks(I + 1, [0, 4])
                    pres[I + 1] = {
                        pr: qk_chunks(I + 1, [pr, 4 + pr])
                        for pr in range(1, PAIRS)
                    }
                attn_block(I, nxt, pre=pres.get(I))
    nc.compile()
    return nc


def get_nc():
    if "nc" not in _cache:
        _cache["nc"] = _build_nc()
    return _cache["nc"]


def _fp8_pair(a):
    import ml_dtypes

    E4 = ml_dtypes.float8_e4m3
    hi = a.astype(E4)
    lo = (a - hi.astype(np.float32)).astype(E4)
    return hi, lo


def shard_inputs(x, w_attn, b_attn):
    """Full inputs -> per-core input maps (host-side quantize/slice)."""
    x = np.asarray(x, dtype=np.float32)
    w = np.asarray(w_attn, dtype=np.float32) * WSCALE
    bb = np.asarray(b_attn, dtype=np.float32) * WSCALE
    x8h, x8l = _fp8_pair(x)  # [B, T, E] fp8, quantized once
    in_maps = []
    for core in range(N_CORES):
        b, hg = core // 2, core % 2
        r0 = hg * CC  # first q row for this head group
        # head-pair-major column packing: pr*256+[0:128]=q(pr), +[128:256]=k(pr)
        wq = w[r0 : r0 + CC, :]
        wk = w[C + r0 : C + r0 + CC, :]
        w_qk = np.concatenate(
            sum(
                (
                    [wq[pr * 128 : (pr + 1) * 128], wk[pr * 128 : (pr + 1) * 128]]
                    for pr in range(PAIRS)
                ),
                [],
            ),
            axis=0,
        ).T  # [E, 2CC]
        wqk8h, wqk8l = _fp8_pair(np.ascontiguousarray(w_qk))
        w_v = np.ascontiguousarray(w[2 * C + r0 : 2 * C + r0 + CC, :].T)
        wv8h, wv8l = _fp8_pair(w_v)
        b_qk = np.stack(
            [bb[r0 + ct * 128 : r0 + (ct + 1) * 128] for ct in range(4)]
            + [bb[C + r0 + ct * 128 : C + r0 + (ct + 1) * 128] for ct in range(4)],
            axis=1,
        ).astype(np.float32)
        b_v = bb[2 * C + r0 : 2 * C + r0 + CC].reshape(1, CC).astype(np.float32)
        in_maps.append(
            {
                "xT8h": np.ascontiguousarray(x8h[b].T),
                "xT8l": np.ascontiguousarray(x8l[b].T),
                "wqk8h": wqk8h,
                "wqk8l": wqk8l,
                "wv8h": wv8h,
                "wv8l": wv8l,
                "b_qk": np.ascontiguousarray(b_qk),
                "b_v": np.ascontiguousarray(b_v),
                "ones_d": np.ones((1, 128), dtype=np.float32),
            }
        )
    return in_maps


def run(in_maps, trace=False, **kw):
    from concourse import bass_utils

    nc = get_nc()
    return bass_utils.run_bass_kernel_spmd(
        nc, in_maps, core_ids=list(range(N_CORES)), trace=trace, **kw
    )


def gather_output(results):
    y = np.empty((B, T, E), dtype=np.float32)
    for core in range(N_CORES):
        b, hg = core // 2, core % 2
        y[b, :, hg * CC : (hg + 1) * CC] = results[core]["y_out"].reshape(T, CC)
    return y


def kernel(x, w_attn, b_attn):
    in_maps = shard_inputs(x, w_attn, b_attn)
    res = run(in_maps, trace=False)
    return gather_output(res.results)


# revision 5
# speedup vs baseline: 1.0752x; 1.0752x over previous
"""Causal self-attention (B=4, T=2048, E=1024, H=16) on 8 trn2 NeuronCores.

Sharding: core c -> (batch b = c // 2, head-group hg = c % 2); each core owns
one batch element and 8 of the 16 heads (data parallel on B, tensor parallel
on heads).  No cross-core communication.

v3 design (compensated-fp8 projection + exp split across ScalarE/DVE):
  - QKV projection in fp8e4m3 DoubleRow perf mode: x and w are split hi/lo
    (w pre-scaled x32 on host; lo = fp8(residual), unscaled so both terms
    accumulate in one psum group).  psum = x8h*w8h + x8h*w8l + x8l*w8h over
    4 e-tile pairs = 12 DR matmuls per 128-col group (2.66x fewer PE cycles
    than the f32r version at ~0.12% element error).  q/k bias (x32) added on
    DVE during the psum->sbuf bf16 copy; v bias via a ones-row matmul.
  - QK: bf16 row-tiled head pairs (2 heads per pass), causal trim per
    j-tile; scores land in psum as 1024*score (both q,k carry x32).
  - exp: scale 0.125/1024 on ScalarE; a fraction of the full (non-diagonal)
    j-tiles instead run a 2-instruction custom-DVE exp (deg-4 Horner 16th
    root + 4 squarings, validated at the bf16-output error floor) to
    offload the ScalarE bottleneck.  Causal mask via gpsimd affine_select
    on the diagonal band only (diagonal tiles always take the ScalarE path).
  - PV flipped: pt stationary, v moving ([j, d+ones] 65 cols).  The ones
    column is 32.0 so the denominator matches v's x32 scale and the
    normalization (reciprocal + per-partition-scalar multiply) cancels it.
  - Output normalization multiplies run on gpsimd (Pool) to unload DVE.
  - Output written as y[t, c]; host concatenates without transposing.
"""

import sys

sys.path.insert(0, "/opt/trn_rl_repo")

import numpy as np

N_CORES = 8
B, T, E = 4, 2048, 1024
H, D = 16, 64
C = E                 # q/k/v channel count (4th qkv chunk unused)
HPC = H // 2          # heads per core
CC = HPC * D          # per-core channels = 512
EP = 4                # e-tile pairs (contraction 1024 = 4 pairs x 256)
TB = T // 512         # 4 t/i blocks of 512
NJ = T // 128         # 16 j-tiles of 128
PAIRS = HPC // 2      # 4 head pairs per core

WSCALE = 32.0         # host pre-scale on w/b so fp8 hi/lo are well-ranged

_cache = {}

# ---- custom DVE exp ops ---------------------------------------------------

_EXP_OPS = {}


def _register_exp_ops():
    if _EXP_OPS:
        return _EXP_OPS["p4"], _EXP_OPS["sq4"]
    import re

    import concourse.dve_ops as dops
    from concourse.dve_ops import DveOp
    from concourse.dve_spec import C0, C1, C2, One, Spec, Src0, Src1, sq

    s = Src0
    body_p4 = ((((Src1 * s + C0) * s + C1) * s + C2) * s + One)

    def _ref_p4(in0, in1, s0, s1, imm2):
        r = ((((in1 * in0 + s0) * in0 + s1) * in0 + imm2) * in0 + 1.0)
        return r.astype(np.float32)

    x2 = sq(Src0)
    x4 = sq(x2)
    x8 = sq(x4)
    body_sq4 = sq(x8)

    def _ref_sq4(in0, in1, s0, s1, imm2):
        return ((((in0 * in0) ** 2) ** 2) ** 2).astype(np.float32)

    def make(name, body, ref, row):
        dops._SUB_OPCODE_FOR_NAME[name] = row
        op = DveOp(name, Spec(body=body, reference=ref), subdim=False,
                   uops_sha={"v3": "?"})
        try:
            op.compile("v3")
        except ValueError as e:
            m = re.search(r"v3: ([0-9a-f]+) ", str(e))
            assert m, f"cannot parse uops sha from: {e}"
            dops._COMPILE_CACHE.pop((name, "v3"), None)
            object.__setattr__(op, "uops_sha", {"v3": m.group(1)})
        op.compile("v3")
        dops.OPS.append(op)
        dops.CUSTOM_DVE_SPECS[name] = op.spec
        return op

    row = max(dops._SUB_OPCODE_FOR_NAME.values())
    assert row + 2 < 0x20, "custom-DVE row field overflow"
    p4 = make("EXP_P4_ANT", body_p4, _ref_p4, row + 1)
    sq4 = make("EXP_SQ4_ANT", body_sq4, _ref_sq4, row + 2)
    _EXP_OPS.update(p4=p4, sq4=sq4)
    return p4, sq4


def _exp_coeffs():
    """P(u) = (((a u + b) u + c) u + d) u + 1 with P(K*st)^16 ~ exp(K16*st),
    K16 = 0.125/1024 (st holds 1024*score, exp wants 0.125*score).  Fit the
    16th root e^(u/16) by Chebyshev interpolation over u in [-6.5, 6.5]
    (covers |score| <= 6.5), normalize constant term to 1 (the resulting
    global e16 factor cancels in the softmax normalization)."""
    u0, n = 6.5, 5
    xk = np.cos(np.pi * (2 * np.arange(n) + 1) / (2 * n)) * u0
    V = np.vander(xk, n, increasing=True)
    coef = np.linalg.solve(V, np.exp(xk / 16.0))
    K = 0.125 / 1024.0
    coef = coef * (K ** np.arange(n))
    c0, c1, c2, c3, c4 = [float(v) for v in coef]
    return c4 / c0, c3 / c0, c2 / c0, c1 / c0


def _build_nc():
    import concourse.mybir as mybir
    import concourse.tile as tile
    from concourse import bacc

    p4_op, sq4_op = _register_exp_ops()
    CA, CB, CCo, CD = _exp_coeffs()

    f32 = mybir.dt.float32
    f32r = mybir.dt.float32r
    bf16 = mybir.dt.bfloat16
    fp8 = mybir.dt.float8e4
    Act = mybir.ActivationFunctionType
    DR = mybir.MatmulPerfMode.DoubleRow
    is_ge = mybir.AluOpType.is_ge

    nc = bacc.Bacc("TRN2", target_bir_lowering=False, debug=False)

    xT8h = nc.dram_tensor("xT8h", [E, T], fp8, kind="ExternalInput").ap()
    xT8l = nc.dram_tensor("xT8l", [E, T], fp8, kind="ExternalInput").ap()
    wqk8h = nc.dram_tensor("wqk8h", [E, 2 * CC], fp8, kind="ExternalInput").ap()
    wqk8l = nc.dram_tensor("wqk8l", [E, 2 * CC], fp8, kind="ExternalInput").ap()
    wv8h = nc.dram_tensor("wv8h", [E, CC], fp8, kind="ExternalInput").ap()
    wv8l = nc.dram_tensor("wv8l", [E, CC], fp8, kind="ExternalInput").ap()
    b_qk = nc.dram_tensor("b_qk", [128, 8], f32, kind="ExternalInput").ap()
    b_v = nc.dram_tensor("b_v", [1, CC], f32r, kind="ExternalInput").ap()
    ones_d = nc.dram_tensor("ones_d", [1, 128], f32r, kind="ExternalInput").ap()
    # flat [(I, it, p), (head, d)] == [T, CC] row-major; out DMAs use strided
    # APs so (head, d) runs stay 512B-contiguous.
    y_out = nc.dram_tensor("y_out", [T, CC], f32, kind="ExternalOutput").ap()

    def pair_rows(dram, k, csl=None):
        """[E, N] dram rows 256k..256k+255 -> [128, 2, n] AP (e-pair dim1)."""
        sl = dram[256 * k : 256 * (k + 1), :] if csl is None else \
            dram[256 * k : 256 * (k + 1), csl]
        return sl.rearrange("(two p) n -> p two n", two=2)

    with tile.TileContext(nc) as tc:
        with (
            tc.tile_pool(name="persist", bufs=1) as pp,
            tc.tile_pool(name="psum", bufs=1, space="PSUM") as psp,
            tc.tile_pool(name="xpool", bufs=2) as xp,
            tc.tile_pool(name="ptpool", bufs=12) as ptp,
            tc.tile_pool(name="upool", bufs=3) as up,
            tc.tile_pool(name="opool", bufs=1) as op,
        ):
            # ---- persistent SBUF state ----
            qk_sb = [pp.tile([128, T], bf16, name=f"qk{ct}") for ct in range(8)]
            # v plus a 32.0 column per head: [t-part, head, j-tile, 65]
            v1_sb = pp.tile([128, HPC, NJ, D + 1], bf16, name="v1")
            bqk_sb = pp.tile([128, 8], f32, name="bqk")
            bv_sb = pp.tile([1, CC], f32r, name="bv")
            ones_sb = pp.tile([1, 128], f32r, name="ones")
            acoef_sb = pp.tile([128, 1024], f32, name="acoef")
            wqh_t = []
            wql_t = []
            wvh_t = []
            wvl_t = []

            # softmax-denominator column: 32.0 matches v's x32 scale so the
            # per-row normalization cancels it exactly
            nc.gpsimd.memset(v1_sb[:, :, :, D : D + 1], WSCALE)
            nc.gpsimd.memset(acoef_sb, CA)

            xs_tb = {}

            def load_x(tb):
                tsl = slice(tb * 512, (tb + 1) * 512)
                xs = []
                for k in range(EP):
                    xh = xp.tile([128, 2, 512], fp8, tag=f"xh{k}", bufs=2,
                                 name=f"xh{k}_{tb}")
                    nc.sync.dma_start(out=xh, in_=pair_rows(xT8h, k, tsl))
                    xl = xp.tile([128, 2, 512], fp8, tag=f"xl{k}", bufs=2,
                                 name=f"xl{k}_{tb}")
                    nc.sync.dma_start(out=xl, in_=pair_rows(xT8l, k, tsl))
                    xs.append((xh, xl))
                xs_tb[tb] = xs

            # small constants, then x0/wqk interleaved per e-pair (the
            # exp-critical path: pair 0's q/k projection), then wv
            nc.sync.dma_start(out=bqk_sb, in_=b_qk)
            nc.sync.dma_start(out=bv_sb, in_=b_v)
            nc.sync.dma_start(out=ones_sb, in_=ones_d)
            tsl0 = slice(0, 512)
            xs0 = []
            # (host packs w_qk cols pr-major: pr*256+[0:128]=q, +[128:256]=k)
            for k in range(EP):
                xh = xp.tile([128, 2, 512], fp8, tag=f"xh{k}", bufs=2,
                             name=f"xh{k}_0")
                nc.sync.dma_start(out=xh, in_=pair_rows(xT8h, k, tsl0))
                xl = xp.tile([128, 2, 512], fp8, tag=f"xl{k}", bufs=2,
                             name=f"xl{k}_0")
                nc.sync.dma_start(out=xl, in_=pair_rows(xT8l, k, tsl0))
                xs0.append((xh, xl))
                # cols 0:256 = pair-0's q and k — the exp-critical path
                wqh = pp.tile([128, 2, 2 * CC], fp8, name=f"wqh{k}")
                nc.sync.dma_start(out=wqh[:, :, 0:256],
                                  in_=pair_rows(wqk8h, k, slice(0, 256)))
                wql = pp.tile([128, 2, 2 * CC], fp8, name=f"wql{k}")
                nc.sync.dma_start(out=wql[:, :, 0:256],
                                  in_=pair_rows(wqk8l, k, slice(0, 256)))
                wqh_t.append(wqh)
                wql_t.append(wql)
            for k in range(EP):
                nc.sync.dma_start(out=wqh_t[k][:, :, 256:512],
                                  in_=pair_rows(wqk8h, k, slice(256, 512)))
                nc.sync.dma_start(out=wql_t[k][:, :, 256:512],
                                  in_=pair_rows(wqk8l, k, slice(256, 512)))
            xs_tb[0] = xs0
            for k in range(EP):
                wvh = pp.tile([128, 2, CC], fp8, name=f"wvh{k}")
                nc.sync.dma_start(out=wvh, in_=pair_rows(wv8h, k))
                wvl = pp.tile([128, 2, CC], fp8, name=f"wvl{k}")
                nc.sync.dma_start(out=wvl, in_=pair_rows(wv8l, k))
                wvh_t.append(wvh)
                wvl_t.append(wvl)
            for k in range(EP):
                nc.sync.dma_start(out=wqh_t[k][:, :, 512:1024],
                                  in_=pair_rows(wqk8h, k, slice(512, 1024)))
                nc.sync.dma_start(out=wql_t[k][:, :, 512:1024],
                                  in_=pair_rows(wqk8l, k, slice(512, 1024)))

            def qk_terms(ct):
                co = (ct % 4) * 256 + (128 if ct >= 4 else 0)
                csl = slice(co, co + 128)
                terms = []
                for k in range(EP):
                    xh, xl = None, None  # bound at emit time via xs_tb
                    terms.append((k, "hh", csl))
                    terms.append((k, "hl", csl))
                    terms.append((k, "lh", csl))
                return terms

            def qkv_group_qk(tb, ct, lo=0, hi=12, cell=None):
                """Emit DR-term chunk [lo, hi) of the ct projection group;
                the last chunk appends the DVE bias-add."""
                tsl = slice(tb * 512, (tb + 1) * 512)
                xs = xs_tb[tb]
                terms = qk_terms(ct)
                if cell is None:
                    cell = {}
                if lo == 0:
                    cell["ps"] = psp.tile([128, 512], f32, tag="qp", bufs=2,
                                          name=f"psqk{ct}_{tb}")
                ps = cell["ps"]
                for i in range(lo, hi):
                    k, kind, csl = terms[i]
                    xh, xl = xs[k]
                    w = (wqh_t if kind[0] == "h" else wql_t)[k][:, :, csl]
                    x = xh if kind[1] == "h" else xl
                    nc.tensor.matmul(
                        ps, w, x,
                        start=(i == 0),
                        stop=(i == 11),
                        perf_mode=DR,
                        skip_group_check=True,
                    )
                if hi == 12:
                    # bias add on DVE (psum f32 + [128,1] bias -> sbuf bf16)
                    nc.vector.tensor_scalar_add(
                        qk_sb[ct][:, tsl], ps, bqk_sb[:, ct : ct + 1])

            def qkv_group_v(tb, k4, lo=0, hi=12, cell=None):
                xs = xs_tb[tb]
                tt = tb * 4 + k4
                csl = slice(k4 * 128, (k4 + 1) * 128)
                if cell is None:
                    cell = {}
                if lo == 0:
                    cell["ps"] = psp.tile([128, 512], f32, tag="qp", bufs=2,
                                          name=f"psv{tt}")
                    nc.tensor.matmul(
                        cell["ps"], ones_sb, bv_sb,
                        start=True, stop=False, skip_group_check=True,
                    )
                psv = cell["ps"]
                terms = [(k, kind) for k in range(EP)
                         for kind in ("hh", "hl", "lh")]
                for i in range(lo, hi):
                    k, kind = terms[i]
                    xh, xl = xs[k]
                    x = (xh if kind[1] == "h" else xl)[:, :, csl]
                    w = (wvh_t if kind[0] == "h" else wvl_t)[k]
                    nc.tensor.matmul(
                        psv, x, w,
                        start=False,
                        stop=(i == 11),
                        perf_mode=DR,
                        skip_group_check=True,
                    )
                if hi == 12:
                    nc.vector.tensor_copy(
                        v1_sb[:, :, tt, 0:D],
                        psv.rearrange("p (h d) -> p h d", d=D),
                    )

            def qk_chunks(tb, cts):
                steps = []
                for ct in cts:
                    cell = {}
                    for lo, hi in ((0, 4), (4, 8), (8, 12)):
                        steps.append(
                            lambda c=ct, l=lo, h=hi, ce=cell:
                            qkv_group_qk(tb, c, l, h, ce))
                return steps

            def v_chunks(tb):
                steps = []
                for g in range(4):
                    cell = {}
                    for lo, hi in ((0, 4), (4, 8), (8, 12)):
                        steps.append(
                            lambda k=g, l=lo, h=hi, ce=cell:
                            qkv_group_v(tb, k, l, h, ce))
                return steps

            def attn_block(I, nxt=(), pre=None):
                nj = 4 * I + 4  # causal j-tiles for this i-block
                yts = {}
                pts = {}
                ndve = {}

                def qk_exp(pr, J):
                    r = J - 4 * I
                    ws = 128 * r if r > 0 else 0  # causal trim offset
                    qt = qk_sb[pr]
                    kt = qk_sb[4 + pr]
                    jsl = slice(J * 128, (J + 1) * 128)
                    iwl = slice(I * 512 + ws, (I + 1) * 512)
                    st = psp.tile([128, 2, 512], f32, tag="st", bufs=2,
                                  name=f"st{pr}_{I}_{J}")
                    # QK row-tile pair: head A rows 0-63, head B 64-127
                    nc.tensor.matmul(
                        st[:, 0, ws:], kt[0:64, jsl], qt[0:64, iwl],
                        tile_position=(0, 0),
                    )
                    nc.tensor.matmul(
                        st[:, 1, ws:], kt[64:128, jsl], qt[64:128, iwl],
                        tile_position=(64, 0),
                    )
                    pt = ptp.tile([128, 2, 512], bf16, tag="pt",
                                  name=f"pt{pr}_{I}_{J}")
                    # route a fraction of full tiles through the DVE exp to
                    # unload ScalarE; only in late blocks, where ScalarE is
                    # saturated and DVE is idle (early blocks are PE-paced:
                    # ScalarE starves anyway, and DVE is busy with the
                    # projection copies).  Diagonal tiles (r >= 0, need
                    # masking/trim) always use ScalarE.
                    mod = {2: 3, 3: 3}.get(I, 0)
                    use_dve = (r < 0) and mod and (ndve["n"] % mod == 1)
                    if r < 0:
                        ndve["n"] += 1
                    if use_dve:
                        u = up.tile([128, 1024], f32, tag="u",
                                    name=f"u{pr}_{I}_{J}")
                        nc.vector._custom_dve(
                            p4_op, out=u,
                            in0=st.rearrange("p h w -> p (h w)"),
                            in1=acoef_sb, s0=CB, s1=CCo, imm2=CD)
                        nc.vector._custom_dve(
                            sq4_op, out=pt.rearrange("p h w -> p (h w)"),
                            in0=u)
                    else:
                        nc.scalar.activation(pt[:, :, ws:], st[:, :, ws:],
                                             Act.Exp, scale=0.125 / 1024.0)
                    if r >= 0:
                        # causal mask on the 128-wide diagonal band only:
                        # keep where i_band >= j (within-tile coords)
                        nc.gpsimd.affine_select(
                            out=pt[:, :, 128 * r : 128 * (r + 1)],
                            in_=pt[:, :, 128 * r : 128 * (r + 1)],
                            compare_op=is_ge,
                            fill=0.0,
                            base=0,
                            pattern=[[0, 2], [1, 128]],
                            channel_multiplier=-1,
                        )
                    pts[(pr, J)] = pt

                def pv(pr, J):
                    pt = pts.pop((pr, J))
                    ytA, ytB = yts[pr]
                    r = J - 4 * I
                    for h, yt in ((0, ytA), (1, ytB)):
                        for it in range(4):
                            if r > it:
                                continue  # i-tile fully masked for this j
                            # PSUM zeroing is bank-granular: only the FIRST
                            # region of each bank sets start=True
                            nc.tensor.matmul(
                                yt[:, it, 0 : D + 1],
                                pt[:, h, it * 128 : (it + 1) * 128],
                                v1_sb[:, 2 * pr + h, J, :],
                                start=(J == 0 and it == 0),
                                stop=(J == 4 * I + it),
                                skip_group_check=True,
                            )

                def out_stage(pr):
                    ytA, ytB = yts.pop(pr)
                    rec = op.tile([128, 2, 4], f32, tag="rec", bufs=2,
                                  name=f"rec{pr}_{I}")
                    yc = op.tile([128, 2, 4, D + 1], f32, tag="yc", bufs=2,
                                 name=f"yc{pr}_{I}")
                    ys = op.tile([128, 4, 2, D], f32, tag="ys", bufs=2,
                                 name=f"ys{pr}_{I}")
                    # copy psum->sbuf first: releases the yt banks earlier
                    nc.vector.tensor_copy(yc[:, 0, :, :], ytA[:, :, 0 : D + 1])
                    nc.vector.tensor_copy(yc[:, 1, :, :], ytB[:, :, 0 : D + 1])
                    # denominators live in column 64 of each (h, it) slot
                    nc.vector.reciprocal(rec[:, 0, :], yc[:, 0, :, D])
                    nc.vector.reciprocal(rec[:, 1, :], yc[:, 1, :, D])
                    for h in (0, 1):
                        for it in range(4):
                            # per-partition-scalar multiply on gpsimd (Pool)
                            nc.gpsimd.tensor_scalar_mul(
                                ys[:, it, h, :], yc[:, h, it, 0:D],
                                rec[:, h, it : it + 1])
                    # ys [i-part, it, h, d] -> y_out[I, it, p, 2pr+h, d]
                    import concourse.bass as bass

                    out_ap = bass.AP(
                        tensor=y_out.tensor,
                        offset=I * (4 * 128 * CC) + 2 * pr * D,
                        ap=[[CC, 128], [128 * CC, 4], [1, 2 * D]],
                    )
                    nc.sync.dma_start(
                        out=out_ap,
                        in_=ys.rearrange("p i h d -> p i (h d)"),
                    )

                def alloc_yt(pr):
                    # one full 2KB PSUM bank per head so the bank-granular
                    # start=True zeroing touches no other tile
                    yts[pr] = (
                        psp.tile([128, 4, 128], f32, tag="ytA", bufs=1,
                                 name=f"ytA{pr}_{I}"),
                        psp.tile([128, 4, 128], f32, tag="ytB", bufs=1,
                                 name=f"ytB{pr}_{I}"),
                    )

                ndve["n"] = I  # stagger DVE-routed tiles across blocks
                items = [(pr, J) for pr in range(PAIRS) for J in range(nj)]
                nxt = list(nxt)
                nsteps = len(nxt)
                popped = 0
                emitted = 0

                def emit_qk(k):
                    pr, J = items[k]
                    if J == 0:
                        if pre:
                            for fn in pre.get(pr, ()):
                                fn()
                        alloc_yt(pr)
                    qk_exp(pr, J)

                for k in range(len(items)):
                    while emitted < min(k + 4, len(items)):
                        emit_qk(emitted)
                        emitted += 1
                    pr, J = items[k]
                    pv(pr, J)
                    if J == nj - 1:
                        out_stage(pr)
                    # weave next t-block's QKV in small chunks so a long
                    # projection burst never delays the next QK
                    want = (k + 1) * nsteps // len(items)
                    while popped < want:
                        nxt[popped]()
                        popped += 1
                for fn in nxt[popped:]:
                    fn()

            # schedule: per-pair staging for EVERY block.  Block I weaves
            # only what block I+1 needs at its start (v groups + pair 0's
            # q/k); pairs 1-3's q/k groups emit as `pre` inside block I+1.
            qkv_group_qk(0, 0)
            qkv_group_qk(0, 4)
            for g in range(4):
                qkv_group_v(0, g)
            pres = {
                0: {
                    pr: qk_chunks(0, [pr, 4 + pr])
                    for pr in range(1, PAIRS)
                }
            }
            for I in range(TB):
                nxt = []
                if I + 1 < TB:
                    load_x(I + 1)
                    nxt = v_chunks(I + 1) + qk_chunks(I + 1, [0, 4])
                    pres[I + 1] = {
                        pr: qk_chunks(I + 1, [pr, 4 + pr])
                        for pr in range(1, PAIRS)
                    }
                attn_block(I, nxt, pre=pres.get(I))
    nc.compile()
    return nc


def get_nc():
    if "nc" not in _cache:
        _cache["nc"] = _build_nc()
    return _cache["nc"]


def _fp8_pair(a):
    import ml_dtypes

    E4 = ml_dtypes.float8_e4m3
    hi = a.astype(E4)
    lo = (a - hi.astype(np.float32)).astype(E4)
    return hi, lo


def shard_inputs(x, w_attn, b_attn):
    """Full inputs -> per-core input maps (host-side quantize/slice)."""
    x = np.asarray(x, dtype=np.float32)
    w = np.asarray(w_attn, dtype=np.float32) * WSCALE
    bb = np.asarray(b_attn, dtype=np.float32) * WSCALE
    x8h, x8l = _fp8_pair(x)  # [B, T, E] fp8, quantized once
    in_maps = []
    for core in range(N_CORES):
        b, hg = core // 2, core % 2
        r0 = hg * CC  # first q row for this head group
        # head-pair-major column packing: pr*256+[0:128]=q(pr), +[128:256]=k(pr)
        wq = w[r0 : r0 + CC, :]
        wk = w[C + r0 : C + r0 + CC, :]
        w_qk = np.concatenate(
            sum(
                (
                    [wq[pr * 128 : (pr + 1) * 128], wk[pr * 128 : (pr + 1) * 128]]
                    for pr in range(PAIRS)
                ),
                [],
            ),
            axis=0,
        ).T  # [E, 2CC]
        wqk8h, wqk8l = _fp8_pair(np.ascontiguousarray(w_qk))
        w_v = np.ascontiguousarray(w[2 * C + r0 : 2 * C + r0 + CC, :].T)
        wv8h, wv8l = _fp8_pair(w_v)
        b_qk = np.stack(
            [bb[r0 + ct * 128 : r0 + (ct + 1) * 128] for ct in range(4)]
            + [bb[C + r0 + ct * 128 : C + r0 + (ct + 1) * 128] for ct in range(4)],
            axis=1,
        ).astype(np.float32)
        b_v = bb[2 * C + r0 : 2 * C + r0 + CC].reshape(1, CC).astype(np.float32)
        in_maps.append(
            {
                "xT8h": np.ascontiguousarray(x8h[b].T),
                "xT8l": np.ascontiguousarray(x8l[b].T),
                "wqk8h": wqk8h,
                "wqk8l": wqk8l,
                "wv8h": wv8h,
                "wv8l": wv8l,
                "b_qk": np.ascontiguousarray(b_qk),
                "b_v": np.ascontiguousarray(b_v),
                "ones_d": np.ones((1, 128), dtype=np.float32),
            }
        )
    return in_maps


def run(in_maps, trace=False, **kw):
    from concourse import bass_utils

    nc = get_nc()
    return bass_utils.run_bass_kernel_spmd(
        nc, in_maps, core_ids=list(range(N_CORES)), trace=trace, **kw
    )


def gather_output(results):
    y = np.empty((B, T, E), dtype=np.float32)
    for core in range(N_CORES):
        b, hg = core // 2, core % 2
        y[b, :, hg * CC : (hg + 1) * CC] = results[core]["y_out"].reshape(T, CC)
    return y


def kernel(x, w_attn, b_attn):
    in_maps = shard_inputs(x, w_attn, b_attn)
    res = run(in_maps, trace=False)
    return gather_output(res.results)


# revision 13
# speedup vs baseline: 1.1497x; 1.0692x over previous
"""Causal self-attention (B=4, T=2048, E=1024, H=16) on 8 trn2 NeuronCores.

Sharding: core c -> (batch b = c // 2, head-group hg = c % 2); each core owns
one batch element and 8 of the 16 heads (data parallel on B, tensor parallel
on heads).  No cross-core communication.

v3 design (compensated-fp8 projection + exp split across ScalarE/DVE):
  - QKV projection in fp8e4m3 DoubleRow perf mode: x and w are split hi/lo
    (w pre-scaled x32 on host; lo = fp8(residual), unscaled so both terms
    accumulate in one psum group).  psum = x8h*w8h + x8h*w8l + x8l*w8h over
    4 e-tile pairs = 12 DR matmuls per 128-col group (2.66x fewer PE cycles
    than the f32r version at ~0.12% element error).  q/k bias (x32) added on
    DVE during the psum->sbuf bf16 copy; v bias via a ones-row matmul.
  - QK: bf16 row-tiled head pairs (2 heads per pass), causal trim per
    j-tile; scores land in psum as 1024*score (both q,k carry x32).
  - exp: scale 0.125/1024 on ScalarE; a fraction of the full (non-diagonal)
    j-tiles instead run a 2-instruction custom-DVE exp (deg-4 Horner 16th
    root + 4 squarings, validated at the bf16-output error floor) to
    offload the ScalarE bottleneck.  Causal mask via gpsimd affine_select
    on the diagonal band only (diagonal tiles always take the ScalarE path).
  - PV flipped: pt stationary, v moving ([j, d+ones] 65 cols).  The ones
    column is 32.0 so the denominator matches v's x32 scale and the
    normalization (reciprocal + per-partition-scalar multiply) cancels it.
  - Output normalization multiplies run on gpsimd (Pool) to unload DVE.
  - Output written as y[t, c]; host concatenates without transposing.
"""

import sys

sys.path.insert(0, "/opt/trn_rl_repo")

import numpy as np

N_CORES = 8
B, T, E = 4, 2048, 1024
H, D = 16, 64
C = E                 # q/k/v channel count (4th qkv chunk unused)
HPC = H // 2          # heads per core
CC = HPC * D          # per-core channels = 512
EP = 4                # e-tile pairs (contraction 1024 = 4 pairs x 256)
TB = T // 512         # 4 t/i blocks of 512
NJ = T // 128         # 16 j-tiles of 128
PAIRS = HPC // 2      # 4 head pairs per core

WSCALE = 32.0         # host pre-scale on w/b so fp8 hi/lo are well-ranged

_cache = {}

# ---- custom DVE exp ops ---------------------------------------------------

_EXP_OPS = {}


def _register_exp_ops():
    if _EXP_OPS:
        return _EXP_OPS["p4"], _EXP_OPS["sq4"]
    import re

    import concourse.dve_ops as dops
    from concourse.dve_ops import DveOp
    from concourse.dve_spec import C0, C1, C2, One, Spec, Src0, Src1, sq

    s = Src0
    body_p4 = ((((Src1 * s + C0) * s + C1) * s + C2) * s + One)

    def _ref_p4(in0, in1, s0, s1, imm2):
        r = ((((in1 * in0 + s0) * in0 + s1) * in0 + imm2) * in0 + 1.0)
        return r.astype(np.float32)

    x2 = sq(Src0)
    x4 = sq(x2)
    x8 = sq(x4)
    body_sq4 = sq(x8)

    def _ref_sq4(in0, in1, s0, s1, imm2):
        return ((((in0 * in0) ** 2) ** 2) ** 2).astype(np.float32)

    def make(name, body, ref, row):
        dops._SUB_OPCODE_FOR_NAME[name] = row
        op = DveOp(name, Spec(body=body, reference=ref), subdim=False,
                   uops_sha={"v3": "?"})
        try:
            op.compile("v3")
        except ValueError as e:
            m = re.search(r"v3: ([0-9a-f]+) ", str(e))
            assert m, f"cannot parse uops sha from: {e}"
            dops._COMPILE_CACHE.pop((name, "v3"), None)
            object.__setattr__(op, "uops_sha", {"v3": m.group(1)})
        op.compile("v3")
        dops.OPS.append(op)
        dops.CUSTOM_DVE_SPECS[name] = op.spec
        return op

    row = max(dops._SUB_OPCODE_FOR_NAME.values())
    assert row + 2 < 0x20, "custom-DVE row field overflow"
    p4 = make("EXP_P4_ANT", body_p4, _ref_p4, row + 1)
    sq4 = make("EXP_SQ4_ANT", body_sq4, _ref_sq4, row + 2)
    _EXP_OPS.update(p4=p4, sq4=sq4)
    return p4, sq4


def _exp_coeffs():
    """P(u) = (((a u + b) u + c) u + d) u + 1 with P(K*st)^16 ~ exp(K16*st),
    K16 = 0.125/1024 (st holds 1024*score, exp wants 0.125*score).  Fit the
    16th root e^(u/16) by Chebyshev interpolation over u in [-6.5, 6.5]
    (covers |score| <= 6.5), normalize constant term to 1 (the resulting
    global e16 factor cancels in the softmax normalization)."""
    u0, n = 6.5, 5
    xk = np.cos(np.pi * (2 * np.arange(n) + 1) / (2 * n)) * u0
    V = np.vander(xk, n, increasing=True)
    coef = np.linalg.solve(V, np.exp(xk / 16.0))
    K = 0.125 / 1024.0
    coef = coef * (K ** np.arange(n))
    c0, c1, c2, c3, c4 = [float(v) for v in coef]
    return c4 / c0, c3 / c0, c2 / c0, c1 / c0


def _build_nc():
    import concourse.mybir as mybir
    import concourse.tile as tile
    from concourse import bacc

    p4_op, sq4_op = _register_exp_ops()
    CA, CB, CCo, CD = _exp_coeffs()

    f32 = mybir.dt.float32
    f32r = mybir.dt.float32r
    bf16 = mybir.dt.bfloat16
    fp8 = mybir.dt.float8e4
    Act = mybir.ActivationFunctionType
    DR = mybir.MatmulPerfMode.DoubleRow
    is_ge = mybir.AluOpType.is_ge

    nc = bacc.Bacc("TRN2", target_bir_lowering=False, debug=False)

    xT8h = nc.dram_tensor("xT8h", [E, T], fp8, kind="ExternalInput").ap()
    xT8l = nc.dram_tensor("xT8l", [E, T], fp8, kind="ExternalInput").ap()
    wqk8h = nc.dram_tensor("wqk8h", [E, 2 * CC], fp8, kind="ExternalInput").ap()
    wqk8l = nc.dram_tensor("wqk8l", [E, 2 * CC], fp8, kind="ExternalInput").ap()
    wv8h = nc.dram_tensor("wv8h", [E, CC], fp8, kind="ExternalInput").ap()
    wv8l = nc.dram_tensor("wv8l", [E, CC], fp8, kind="ExternalInput").ap()
    b_qk = nc.dram_tensor("b_qk", [128, 8], f32, kind="ExternalInput").ap()
    b_v = nc.dram_tensor("b_v", [1, CC], f32r, kind="ExternalInput").ap()
    ones_d = nc.dram_tensor("ones_d", [1, 128], f32r, kind="ExternalInput").ap()
    # flat [(I, it, p), (head, d)] == [T, CC] row-major; out DMAs use strided
    # APs so (head, d) runs stay 512B-contiguous.
    y_out = nc.dram_tensor("y_out", [T, CC], f32, kind="ExternalOutput").ap()

    def all_pairs(dram, csl=None):
        """[E, N] dram -> [128, pair, two, n] AP (all 4 e-pairs, one DMA)."""
        sl = dram if csl is None else dram[:, csl]
        return sl.rearrange("(k two p) n -> p k two n", two=2, k=EP)

    with tile.TileContext(nc) as tc:
        with (
            tc.tile_pool(name="persist", bufs=1) as pp,
            tc.tile_pool(name="psum", bufs=1, space="PSUM") as psp,
            tc.tile_pool(name="xpool", bufs=2) as xp,
            tc.tile_pool(name="ptpool", bufs=12) as ptp,
            tc.tile_pool(name="upool", bufs=3) as up,
            tc.tile_pool(name="opool", bufs=1) as op,
        ):
            # ---- persistent SBUF state ----
            qk_sb = [pp.tile([128, T], bf16, name=f"qk{ct}") for ct in range(8)]
            # v plus a 32.0 column per head: [t-part, head, j-tile, 65]
            v1_sb = pp.tile([128, HPC, NJ, D + 1], bf16, name="v1")
            bqk_sb = pp.tile([128, 8], f32, name="bqk")
            bv_sb = pp.tile([1, CC], f32r, name="bv")
            ones_sb = pp.tile([1, 128], f32r, name="ones")
            acoef_sb = pp.tile([128, 1024], f32, name="acoef")
            # combined 4-pair weight tiles: [128, pair, two, cols]
            wqh_t = pp.tile([128, EP, 2, 2 * CC], fp8, name="wqh")
            wql_t = pp.tile([128, EP, 2, 2 * CC], fp8, name="wql")
            wvh_t = pp.tile([128, EP, 2, CC], fp8, name="wvh")
            wvl_t = pp.tile([128, EP, 2, CC], fp8, name="wvl")

            # softmax-denominator column: 32.0 matches v's x32 scale so the
            # per-row normalization cancels it exactly
            nc.gpsimd.memset(v1_sb[:, :, :, D : D + 1], WSCALE)
            nc.gpsimd.memset(acoef_sb, CA)

            xs_tb = {}

            def load_x(tb):
                tsl = slice(tb * 512, (tb + 1) * 512)
                xh = xp.tile([128, EP, 2, 512], fp8, tag="xh", bufs=2,
                             name=f"xh_{tb}")
                nc.sync.dma_start(out=xh, in_=all_pairs(xT8h[:, tsl]))
                xl = xp.tile([128, EP, 2, 512], fp8, tag="xl", bufs=2,
                             name=f"xl_{tb}")
                nc.sync.dma_start(out=xl, in_=all_pairs(xT8l[:, tsl]))
                xs_tb[tb] = (xh, xl)

            import os as _os_m
            _env = _os_m.environ
            # small constants, then x0/wqk interleaved per e-pair (the
            # exp-critical path: pair 0's q/k projection), then wv
            nc.sync.dma_start(out=bqk_sb, in_=b_qk)
            nc.sync.dma_start(out=bv_sb, in_=b_v)
            nc.sync.dma_start(out=ones_sb, in_=ones_d)
            # (host packs w_qk cols pr-major: pr*256+[0:128]=q, +[128:256]=k)
            # cols 0:256 = pair-0's q and k — the exp-critical path.  One
            # DMA per tensor/slice: each dma_start costs ~630ns of
            # serialized HWDGE time, while a single large transfer already
            # fans out across the 16 DMA engines.
            nc.sync.dma_start(out=wqh_t[:, :, :, 0:256],
                              in_=all_pairs(wqk8h, slice(0, 256)))
            nc.sync.dma_start(out=wql_t[:, :, :, 0:256],
                              in_=all_pairs(wqk8l, slice(0, 256)))
            load_x(0)
            nc.sync.dma_start(out=wvh_t, in_=all_pairs(wv8h))
            nc.sync.dma_start(out=wvl_t, in_=all_pairs(wv8l))
            nc.sync.dma_start(out=wqh_t[:, :, :, 256:1024],
                              in_=all_pairs(wqk8h, slice(256, 1024)))
            nc.sync.dma_start(out=wql_t[:, :, :, 256:1024],
                              in_=all_pairs(wqk8l, slice(256, 1024)))

            def qk_terms(ct):
                co = (ct % 4) * 256 + (128 if ct >= 4 else 0)
                csl = slice(co, co + 128)
                terms = []
                for k in range(EP):
                    xh, xl = None, None  # bound at emit time via xs_tb
                    terms.append((k, "hh", csl))
                    terms.append((k, "hl", csl))
                    terms.append((k, "lh", csl))
                return terms

            def qkv_group_qk(tb, ct, lo=0, hi=12, cell=None):
                """Emit DR-term chunk [lo, hi) of the ct projection group;
                the last chunk appends the DVE bias-add."""
                tsl = slice(tb * 512, (tb + 1) * 512)
                xs = xs_tb[tb]
                terms = qk_terms(ct)
                if cell is None:
                    cell = {}
                if lo == 0:
                    cell["ps"] = psp.tile([128, 512], f32, tag="qp", bufs=2,
                                          name=f"psqk{ct}_{tb}")
                ps = cell["ps"]
                xh_all, xl_all = xs
                for i in range(lo, hi):
                    k, kind, csl = terms[i]
                    w = (wqh_t if kind[0] == "h" else wql_t)[:, k, :, csl]
                    x = (xh_all if kind[1] == "h" else xl_all)[:, k, :, :]
                    nc.tensor.matmul(
                        ps, w, x,
                        start=(i == 0),
                        stop=(i == 11),
                        perf_mode=DR,
                        skip_group_check=True,
                    )
                if hi == 12:
                    # bias add on DVE (psum f32 + [128,1] bias -> sbuf bf16)
                    nc.vector.tensor_scalar_add(
                        qk_sb[ct][:, tsl], ps, bqk_sb[:, ct : ct + 1])

            def qkv_group_v(tb, k4, lo=0, hi=12, cell=None):
                xs = xs_tb[tb]
                tt = tb * 4 + k4
                csl = slice(k4 * 128, (k4 + 1) * 128)
                if cell is None:
                    cell = {}
                if lo == 0:
                    cell["ps"] = psp.tile([128, 512], f32, tag="qp", bufs=2,
                                          name=f"psv{tt}")
                    nc.tensor.matmul(
                        cell["ps"], ones_sb, bv_sb,
                        start=True, stop=False, skip_group_check=True,
                    )
                psv = cell["ps"]
                xh_all, xl_all = xs
                terms = [(k, kind) for k in range(EP)
                         for kind in ("hh", "hl", "lh")]
                for i in range(lo, hi):
                    k, kind = terms[i]
                    x = (xh_all if kind[1] == "h" else xl_all)[:, k, :, csl]
                    w = (wvh_t if kind[0] == "h" else wvl_t)[:, k, :, :]
                    nc.tensor.matmul(
                        psv, x, w,
                        start=False,
                        stop=(i == 11),
                        perf_mode=DR,
                        skip_group_check=True,
                    )
                if hi == 12:
                    nc.vector.tensor_copy(
                        v1_sb[:, :, tt, 0:D],
                        psv.rearrange("p (h d) -> p h d", d=D),
                    )

            def qk_chunks(tb, cts):
                steps = []
                for ct in cts:
                    cell = {}
                    for lo, hi in ((0, 4), (4, 8), (8, 12)):
                        steps.append(
                            lambda c=ct, l=lo, h=hi, ce=cell:
                            qkv_group_qk(tb, c, l, h, ce))
                return steps

            def v_chunks(tb):
                steps = []
                for g in range(4):
                    cell = {}
                    for lo, hi in ((0, 4), (4, 8), (8, 12)):
                        steps.append(
                            lambda k=g, l=lo, h=hi, ce=cell:
                            qkv_group_v(tb, k, l, h, ce))
                return steps

            def attn_block(I, nxt=(), pre=None):
                nj = 4 * I + 4  # causal j-tiles for this i-block
                yts = {}
                pts = {}
                ndve = {}
                dve_set = set()

                def qk_exp(pr, J):
                    r = J - 4 * I
                    ws = 128 * r if r > 0 else 0  # causal trim offset
                    qt = qk_sb[pr]
                    kt = qk_sb[4 + pr]
                    jsl = slice(J * 128, (J + 1) * 128)
                    iwl = slice(I * 512 + ws, (I + 1) * 512)
                    st = psp.tile([128, 2, 512], f32, tag="st", bufs=2,
                                  name=f"st{pr}_{I}_{J}")
                    # QK row-tile pair: head A rows 0-63, head B 64-127
                    nc.tensor.matmul(
                        st[:, 0, ws:], kt[0:64, jsl], qt[0:64, iwl],
                        tile_position=(0, 0),
                    )
                    nc.tensor.matmul(
                        st[:, 1, ws:], kt[64:128, jsl], qt[64:128, iwl],
                        tile_position=(64, 0),
                    )
                    pt = ptp.tile([128, 2, 512], bf16, tag="pt",
                                  name=f"pt{pr}_{I}_{J}")
                    # route a fraction of full tiles through the DVE exp to
                    # unload ScalarE; only in late blocks, where ScalarE is
                    # saturated and DVE is idle (early blocks are PE-paced:
                    # ScalarE starves anyway, and DVE is busy with the
                    # projection copies).  Diagonal tiles (r >= 0, need
                    # masking/trim) always use ScalarE.
                    import os
                    _m = os.environ.get('DVE_MOD', '0:0,1:5,2:4,3:4')
                    _mm = {int(a): int(b) for a, b in (p.split(':') for p in _m.split(','))}
                    mod = _mm.get(I, 0)
                    import os as _os_g
                    _hi = 4 * I - 3 if _os_g.environ.get("DVE_GUARD", "1") == "1" else 4 * I - 1
                    use_dve = (1 <= J <= _hi) and mod and \
                        (ndve["n"] % mod == 1)
                    if r < 0:
                        ndve["n"] += 1
                    if use_dve:
                        dve_set.add((pr, J))
                    if use_dve:
                        u = up.tile([128, 1024], f32, tag="u",
                                    name=f"u{pr}_{I}_{J}")
                        nc.vector._custom_dve(
                            p4_op, out=u,
                            in0=st.rearrange("p h w -> p (h w)"),
                            in1=acoef_sb, s0=CB, s1=CCo, imm2=CD)
                        nc.vector._custom_dve(
                            sq4_op, out=pt.rearrange("p h w -> p (h w)"),
                            in0=u)
                    else:
                        nc.scalar.activation(pt[:, :, ws:], st[:, :, ws:],
                                             Act.Exp, scale=0.125 / 1024.0)
                    if r >= 0:
                        # causal mask on the 128-wide diagonal band only:
                        # keep where i_band >= j (within-tile coords)
                        nc.gpsimd.affine_select(
                            out=pt[:, :, 128 * r : 128 * (r + 1)],
                            in_=pt[:, :, 128 * r : 128 * (r + 1)],
                            compare_op=is_ge,
                            fill=0.0,
                            base=0,
                            pattern=[[0, 2], [1, 128]],
                            channel_multiplier=-1,
                        )
                    pts[(pr, J)] = pt

                def pv(pr, J):
                    pt = pts.pop((pr, J))
                    ytA, ytB = yts[pr]
                    r = J - 4 * I
                    for h, yt in ((0, ytA), (1, ytB)):
                        for it in range(4):
                            if r > it:
                                continue  # i-tile fully masked for this j
                            # PSUM zeroing is bank-granular: only the FIRST
                            # region of each bank sets start=True
                            nc.tensor.matmul(
                                yt[:, it, 0 : D + 1],
                                pt[:, h, it * 128 : (it + 1) * 128],
                                v1_sb[:, 2 * pr + h, J, :],
                                start=(J == 0 and it == 0),
                                stop=(J == 4 * I + it),
                                skip_group_check=True,
                            )

                def out_stage(pr):
                    ytA, ytB = yts.pop(pr)
                    rec = op.tile([128, 2, 4], f32, tag="rec", bufs=2,
                                  name=f"rec{pr}_{I}")
                    yc = op.tile([128, 2, 4, D + 1], f32, tag="yc", bufs=2,
                                 name=f"yc{pr}_{I}")
                    ys = op.tile([128, 4, 2, D], f32, tag="ys", bufs=2,
                                 name=f"ys{pr}_{I}")
                    # copy psum->sbuf first: releases the yt banks earlier
                    nc.vector.tensor_copy(yc[:, 0, :, :], ytA[:, :, 0 : D + 1])
                    nc.vector.tensor_copy(yc[:, 1, :, :], ytB[:, :, 0 : D + 1])
                    # denominators live in column 64 of each (h, it) slot
                    nc.vector.reciprocal(rec[:, 0, :], yc[:, 0, :, D])
                    nc.vector.reciprocal(rec[:, 1, :], yc[:, 1, :, D])
                    for h in (0, 1):
                        for it in range(4):
                            # per-partition-scalar multiply on gpsimd (Pool)
                            nc.gpsimd.tensor_scalar_mul(
                                ys[:, it, h, :], yc[:, h, it, 0:D],
                                rec[:, h, it : it + 1])
                    # ys [i-part, it, h, d] -> y_out[I, it, p, 2pr+h, d]
                    import concourse.bass as bass

                    out_ap = bass.AP(
                        tensor=y_out.tensor,
                        offset=I * (4 * 128 * CC) + 2 * pr * D,
                        ap=[[CC, 128], [128 * CC, 4], [1, 2 * D]],
                    )
                    nc.sync.dma_start(
                        out=out_ap,
                        in_=ys.rearrange("p i h d -> p i (h d)"),
                    )

                def alloc_yt(pr):
                    # one full 2KB PSUM bank per head so the bank-granular
                    # start=True zeroing touches no other tile
                    yts[pr] = (
                        psp.tile([128, 4, 128], f32, tag="ytA", bufs=1,
                                 name=f"ytA{pr}_{I}"),
                        psp.tile([128, 4, 128], f32, tag="ytB", bufs=1,
                                 name=f"ytB{pr}_{I}"),
                    )

                ndve["n"] = I  # stagger DVE-routed tiles across blocks
                items = [(pr, J) for pr in range(PAIRS) for J in range(nj)]
                nxt = list(nxt)
                nsteps = len(nxt)
                popped = 0
                emitted = 0
                pend = []  # deferred (due, pr, J) PVs, kept due-sorted
                left = {pr: nj for pr in range(PAIRS)}

                def emit_qk(k):
                    pr, J = items[k]
                    if J == 0:
                        if pre:
                            for fn in pre.get(pr, ()):
                                fn()
                        alloc_yt(pr)
                    qk_exp(pr, J)

                def run_pv(pr, J):
                    pv(pr, J)
                    left[pr] -= 1
                    if left[pr] == 0:
                        out_stage(pr)

                for k in range(len(items)):
                    while emitted < min(k + 4, len(items)):
                        emit_qk(emitted)
                        emitted += 1
                    # weave next t-block's QKV BEFORE this item's PV: the
                    # chunks hide inside PV's exp-wait window instead of
                    # delaying the next QK (which would starve ScalarE)
                    import os as _os
                    if _os.environ.get("SCHED_WEAVE", "pre") == "pre":
                        want = (k + 1) * nsteps // len(items)
                        while popped < want:
                            nxt[popped]()
                            popped += 1
                    pr, J = items[k]
                    # DVE-routed tiles produce pt ~2us later than ScalarE
                    # ones; defer their PV 2 items so it doesn't head-of-
                    # line-block the in-order PE queue (safe: routing is
                    # restricted to J <= 4I-3, clear of all stop flags)
                    import bisect

                    _defer = int(_os.environ.get("SCHED_DEFER", "2"))
                    due = k + _defer if (pr, J) in dve_set else k
                    bisect.insort(pend, (due, pr, J))
                    while pend and pend[0][0] <= k:
                        _, p2, J2 = pend.pop(0)
                        run_pv(p2, J2)
                    if _os.environ.get("SCHED_WEAVE", "pre") != "pre":
                        want = (k + 1) * nsteps // len(items)
                        while popped < want:
                            nxt[popped]()
                            popped += 1
                for _, p2, J2 in pend:
                    run_pv(p2, J2)
                for fn in nxt[popped:]:
                    fn()

            # schedule: per-pair staging for EVERY block.  Block I weaves
            # only what block I+1 needs at its start (v groups + pair 0's
            # q/k); pairs 1-3's q/k groups emit as `pre` inside block I+1.
            qkv_group_qk(0, 0)
            qkv_group_qk(0, 4)
            for g in range(4):
                qkv_group_v(0, g)
            pres = {
                0: {
                    pr: qk_chunks(0, [pr, 4 + pr])
                    for pr in range(1, PAIRS)
                }
            }
            for I in range(TB):
                nxt = []
                if I + 1 < TB:
                    load_x(I + 1)
                    nxt = v_chunks(I + 1) + qk_chunks(I + 1, [0, 4])
                    pres[I + 1] = {
                        pr: qk_chunks(I + 1, [pr, 4 + pr])
                        for pr in range(1, PAIRS)
                    }
                attn_block(I, nxt, pre=pres.get(I))
    nc.compile()
    return nc


def get_nc():
    if "nc" not in _cache:
        _cache["nc"] = _build_nc()
    return _cache["nc"]


def _fp8_pair(a):
    import ml_dtypes

    E4 = ml_dtypes.float8_e4m3
    hi = a.astype(E4)
    lo = (a - hi.astype(np.float32)).astype(E4)
    return hi, lo


def shard_inputs(x, w_attn, b_attn):
    """Full inputs -> per-core input maps (host-side quantize/slice)."""
    x = np.asarray(x, dtype=np.float32)
    w = np.asarray(w_attn, dtype=np.float32) * WSCALE
    bb = np.asarray(b_attn, dtype=np.float32) * WSCALE
    x8h, x8l = _fp8_pair(x)  # [B, T, E] fp8, quantized once
    in_maps = []
    for core in range(N_CORES):
        b, hg = core // 2, core % 2
        r0 = hg * CC  # first q row for this head group
        # head-pair-major column packing: pr*256+[0:128]=q(pr), +[128:256]=k(pr)
        wq = w[r0 : r0 + CC, :]
        wk = w[C + r0 : C + r0 + CC, :]
        w_qk = np.concatenate(
            sum(
                (
                    [wq[pr * 128 : (pr + 1) * 128], wk[pr * 128 : (pr + 1) * 128]]
                    for pr in range(PAIRS)
                ),
                [],
            ),
            axis=0,
        ).T  # [E, 2CC]
        wqk8h, wqk8l = _fp8_pair(np.ascontiguousarray(w_qk))
        w_v = np.ascontiguousarray(w[2 * C + r0 : 2 * C + r0 + CC, :].T)
        wv8h, wv8l = _fp8_pair(w_v)
        b_qk = np.stack(
            [bb[r0 + ct * 128 : r0 + (ct + 1) * 128] for ct in range(4)]
            + [bb[C + r0 + ct * 128 : C + r0 + (ct + 1) * 128] for ct in range(4)],
            axis=1,
        ).astype(np.float32)
        b_v = bb[2 * C + r0 : 2 * C + r0 + CC].reshape(1, CC).astype(np.float32)
        in_maps.append(
            {
                "xT8h": np.ascontiguousarray(x8h[b].T),
                "xT8l": np.ascontiguousarray(x8l[b].T),
                "wqk8h": wqk8h,
                "wqk8l": wqk8l,
                "wv8h": wv8h,
                "wv8l": wv8l,
                "b_qk": np.ascontiguousarray(b_qk),
                "b_v": np.ascontiguousarray(b_v),
                "ones_d": np.ones((1, 128), dtype=np.float32),
            }
        )
    return in_maps


def run(in_maps, trace=False, **kw):
    from concourse import bass_utils

    nc = get_nc()
    return bass_utils.run_bass_kernel_spmd(
        nc, in_maps, core_ids=list(range(N_CORES)), trace=trace, **kw
    )


def gather_output(results):
    y = np.empty((B, T, E), dtype=np.float32)
    for core in range(N_CORES):
        b, hg = core // 2, core % 2
        y[b, :, hg * CC : (hg + 1) * CC] = results[core]["y_out"].reshape(T, CC)
    return y


def kernel(x, w_attn, b_attn):
    in_maps = shard_inputs(x, w_attn, b_attn)
    res = run(in_maps, trace=False)
    return gather_output(res.results)


# revision 16
# speedup vs baseline: 1.2005x; 1.0442x over previous
"""Causal self-attention (B=4, T=2048, E=1024, H=16) on 8 trn2 NeuronCores.

Sharding: core c -> (batch b = c // 2, head-group hg = c % 2); each core owns
one batch element and 8 of the 16 heads (data parallel on B, tensor parallel
on heads).  No cross-core communication.

v3 design (compensated-fp8 projection + exp split across ScalarE/DVE):
  - QKV projection in fp8e4m3 DoubleRow perf mode: x and w are split hi/lo
    (w pre-scaled x32 on host; lo = fp8(residual), unscaled so both terms
    accumulate in one psum group).  psum = x8h*w8h + x8h*w8l + x8l*w8h over
    4 e-tile pairs = 12 DR matmuls per 128-col group (2.66x fewer PE cycles
    than the f32r version at ~0.12% element error).  q/k bias (x32) added on
    DVE during the psum->sbuf bf16 copy; v bias via a ones-row matmul.
  - QK: bf16 row-tiled head pairs (2 heads per pass), causal trim per
    j-tile; scores land in psum as 1024*score (both q,k carry x32).
  - exp: scale 0.125/1024 on ScalarE; a fraction of the full (non-diagonal)
    j-tiles instead run a 2-instruction custom-DVE exp (deg-4 Horner 16th
    root + 4 squarings, validated at the bf16-output error floor) to
    offload the ScalarE bottleneck.  Causal mask via gpsimd affine_select
    on the diagonal band only (diagonal tiles always take the ScalarE path).
  - PV flipped: pt stationary, v moving ([j, d+ones] 65 cols).  The ones
    column is 32.0 so the denominator matches v's x32 scale and the
    normalization (reciprocal + per-partition-scalar multiply) cancels it.
  - Output normalization multiplies run on gpsimd (Pool) to unload DVE.
  - Output written as y[t, c]; host concatenates without transposing.
"""

import sys

sys.path.insert(0, "/opt/trn_rl_repo")

import numpy as np

N_CORES = 8
B, T, E = 4, 2048, 1024
H, D = 16, 64
C = E                 # q/k/v channel count (4th qkv chunk unused)
HPC = H // 2          # heads per core
CC = HPC * D          # per-core channels = 512
EP = 4                # e-tile pairs (contraction 1024 = 4 pairs x 256)
TB = T // 512         # 4 t/i blocks of 512
NJ = T // 128         # 16 j-tiles of 128
PAIRS = HPC // 2      # 4 head pairs per core

WSCALE = 32.0         # host pre-scale on w/b so fp8 hi/lo are well-ranged

_cache = {}

# ---- custom DVE exp ops ---------------------------------------------------

_EXP_OPS = {}


def _register_exp_ops():
    if _EXP_OPS:
        return _EXP_OPS["p4"], _EXP_OPS["sq4"]
    import re

    import concourse.dve_ops as dops
    from concourse.dve_ops import DveOp
    from concourse.dve_spec import C0, C1, C2, One, Spec, Src0, Src1, sq

    s = Src0
    body_p4 = ((((Src1 * s + C0) * s + C1) * s + C2) * s + One)

    def _ref_p4(in0, in1, s0, s1, imm2):
        r = ((((in1 * in0 + s0) * in0 + s1) * in0 + imm2) * in0 + 1.0)
        return r.astype(np.float32)

    x2 = sq(Src0)
    x4 = sq(x2)
    x8 = sq(x4)
    body_sq4 = sq(x8)

    def _ref_sq4(in0, in1, s0, s1, imm2):
        return ((((in0 * in0) ** 2) ** 2) ** 2).astype(np.float32)

    def make(name, body, ref, row):
        dops._SUB_OPCODE_FOR_NAME[name] = row
        op = DveOp(name, Spec(body=body, reference=ref), subdim=False,
                   uops_sha={"v3": "?"})
        try:
            op.compile("v3")
        except ValueError as e:
            m = re.search(r"v3: ([0-9a-f]+) ", str(e))
            assert m, f"cannot parse uops sha from: {e}"
            dops._COMPILE_CACHE.pop((name, "v3"), None)
            object.__setattr__(op, "uops_sha", {"v3": m.group(1)})
        op.compile("v3")
        dops.OPS.append(op)
        dops.CUSTOM_DVE_SPECS[name] = op.spec
        return op

    row = max(dops._SUB_OPCODE_FOR_NAME.values())
    assert row + 2 < 0x20, "custom-DVE row field overflow"
    p4 = make("EXP_P4_ANT", body_p4, _ref_p4, row + 1)
    sq4 = make("EXP_SQ4_ANT", body_sq4, _ref_sq4, row + 2)
    _EXP_OPS.update(p4=p4, sq4=sq4)
    return p4, sq4


def _exp_coeffs():
    """P(u) = (((a u + b) u + c) u + d) u + 1 with P(K*st)^16 ~ exp(K16*st),
    K16 = 0.125/1024 (st holds 1024*score, exp wants 0.125*score).  Fit the
    16th root e^(u/16) by Chebyshev interpolation over u in [-6.5, 6.5]
    (covers |score| <= 6.5), normalize constant term to 1 (the resulting
    global e16 factor cancels in the softmax normalization)."""
    u0, n = 6.5, 5
    xk = np.cos(np.pi * (2 * np.arange(n) + 1) / (2 * n)) * u0
    V = np.vander(xk, n, increasing=True)
    coef = np.linalg.solve(V, np.exp(xk / 16.0))
    K = 0.125 / 1024.0
    coef = coef * (K ** np.arange(n))
    c0, c1, c2, c3, c4 = [float(v) for v in coef]
    return c4 / c0, c3 / c0, c2 / c0, c1 / c0


def _build_nc():
    import concourse.mybir as mybir
    import concourse.tile as tile
    from concourse import bacc

    p4_op, sq4_op = _register_exp_ops()
    CA, CB, CCo, CD = _exp_coeffs()

    f32 = mybir.dt.float32
    f32r = mybir.dt.float32r
    bf16 = mybir.dt.bfloat16
    fp8 = mybir.dt.float8e4
    Act = mybir.ActivationFunctionType
    DR = mybir.MatmulPerfMode.DoubleRow
    is_ge = mybir.AluOpType.is_ge

    nc = bacc.Bacc("TRN2", target_bir_lowering=False, debug=False)

    xT8h = nc.dram_tensor("xT8h", [E, T], fp8, kind="ExternalInput").ap()
    xT8l = nc.dram_tensor("xT8l", [E, T], fp8, kind="ExternalInput").ap()
    wqk8h = nc.dram_tensor("wqk8h", [E, 2 * CC], fp8, kind="ExternalInput").ap()
    wqk8l = nc.dram_tensor("wqk8l", [E, 2 * CC], fp8, kind="ExternalInput").ap()
    wv8h = nc.dram_tensor("wv8h", [E, CC], fp8, kind="ExternalInput").ap()
    wv8l = nc.dram_tensor("wv8l", [E, CC], fp8, kind="ExternalInput").ap()
    b_qk = nc.dram_tensor("b_qk", [128, 8], f32, kind="ExternalInput").ap()
    b_v = nc.dram_tensor("b_v", [1, CC], f32r, kind="ExternalInput").ap()
    ones_d = nc.dram_tensor("ones_d", [1, 128], f32r, kind="ExternalInput").ap()
    # flat [(I, it, p), (head, d)] == [T, CC] row-major; out DMAs use strided
    # APs so (head, d) runs stay 512B-contiguous.
    y_out = nc.dram_tensor("y_out", [T, CC], f32, kind="ExternalOutput").ap()

    def all_pairs(dram, csl=None):
        """[E, N] dram -> [128, pair, two, n] AP (all 4 e-pairs, one DMA)."""
        sl = dram if csl is None else dram[:, csl]
        return sl.rearrange("(k two p) n -> p k two n", two=2, k=EP)

    with tile.TileContext(nc) as tc:
        with (
            tc.tile_pool(name="persist", bufs=1) as pp,
            tc.tile_pool(name="psum", bufs=1, space="PSUM") as psp,
            tc.tile_pool(name="xpool", bufs=2) as xp,
            tc.tile_pool(name="ptpool", bufs=12) as ptp,
            tc.tile_pool(name="upool", bufs=3) as up,
            tc.tile_pool(name="opool", bufs=1) as op,
        ):
            # ---- persistent SBUF state ----
            qk_sb = [pp.tile([128, T], bf16, name=f"qk{ct}") for ct in range(8)]
            # v plus a 32.0 column per head: [t-part, head, j-tile, 65]
            v1_sb = pp.tile([128, HPC, NJ, D + 1], bf16, name="v1")
            bqk_sb = pp.tile([128, 8], f32, name="bqk")
            bv_sb = pp.tile([1, CC], f32r, name="bv")
            ones_sb = pp.tile([1, 128], f32r, name="ones")
            acoef_sb = pp.tile([128, 1024], f32, name="acoef")
            # combined 4-pair weight tiles: [128, pair, two, cols]
            wqh_t = pp.tile([128, EP, 2, 2 * CC], fp8, name="wqh")
            wql_t = pp.tile([128, EP, 2, 2 * CC], fp8, name="wql")
            wvh_t = pp.tile([128, EP, 2, CC], fp8, name="wvh")
            wvl_t = pp.tile([128, EP, 2, CC], fp8, name="wvl")

            # softmax-denominator column: 32.0 matches v's x32 scale so the
            # per-row normalization cancels it exactly
            nc.gpsimd.memset(v1_sb[:, :, :, D : D + 1], WSCALE)
            nc.gpsimd.memset(acoef_sb, CA)

            xs_tb = {}

            def load_x(tb):
                tsl = slice(tb * 512, (tb + 1) * 512)
                xh = xp.tile([128, EP, 2, 512], fp8, tag="xh", bufs=2,
                             name=f"xh_{tb}")
                nc.sync.dma_start(out=xh, in_=all_pairs(xT8h[:, tsl]))
                xl = xp.tile([128, EP, 2, 512], fp8, tag="xl", bufs=2,
                             name=f"xl_{tb}")
                nc.sync.dma_start(out=xl, in_=all_pairs(xT8l[:, tsl]))
                xs_tb[tb] = (xh, xl)

            import os as _os_m
            _env = _os_m.environ
            # exp-critical path first: x, then pair-0's q/k weight cols
            # (host packs w_qk cols pr-major: pr*256+[0:128]=q, +[128:256]=k).
            # One DMA per tensor/slice: each dma_start costs ~630ns of
            # serialized HWDGE time, while a single large transfer already
            # fans out across the 16 DMA engines.
            nc.sync.dma_start(out=wqh_t[:, :, :, 0:256],
                              in_=all_pairs(wqk8h, slice(0, 256)))
            nc.sync.dma_start(out=wql_t[:, :, :, 0:256],
                              in_=all_pairs(wqk8l, slice(0, 256)))
            load_x(0)
            nc.sync.dma_start(out=bqk_sb, in_=b_qk)
            nc.sync.dma_start(out=bv_sb, in_=b_v)
            nc.sync.dma_start(out=ones_sb, in_=ones_d)
            nc.sync.dma_start(out=wvh_t, in_=all_pairs(wv8h))
            nc.sync.dma_start(out=wvl_t, in_=all_pairs(wv8l))
            nc.sync.dma_start(out=wqh_t[:, :, :, 256:1024],
                              in_=all_pairs(wqk8h, slice(256, 1024)))
            nc.sync.dma_start(out=wql_t[:, :, :, 256:1024],
                              in_=all_pairs(wqk8l, slice(256, 1024)))

            def qk_terms(ct):
                co = (ct % 4) * 256 + (128 if ct >= 4 else 0)
                csl = slice(co, co + 128)
                # kind-major: all hh terms first, xl-dependent lh terms
                # last — matmuls pipeline as the prologue DMAs land
                return [(k, kind, csl) for kind in ("hh", "hl", "lh")
                        for k in range(EP)]

            def qkv_group_qk(tb, ct, lo=0, hi=12, cell=None):
                """Emit DR-term chunk [lo, hi) of the ct projection group;
                the last chunk appends the DVE bias-add."""
                tsl = slice(tb * 512, (tb + 1) * 512)
                xs = xs_tb[tb]
                terms = qk_terms(ct)
                if cell is None:
                    cell = {}
                if lo == 0:
                    cell["ps"] = psp.tile([128, 512], f32, tag="qp", bufs=2,
                                          name=f"psqk{ct}_{tb}")
                ps = cell["ps"]
                xh_all, xl_all = xs
                for i in range(lo, hi):
                    k, kind, csl = terms[i]
                    w = (wqh_t if kind[0] == "h" else wql_t)[:, k, :, csl]
                    x = (xh_all if kind[1] == "h" else xl_all)[:, k, :, :]
                    nc.tensor.matmul(
                        ps, w, x,
                        start=(i == 0),
                        stop=(i == 11),
                        perf_mode=DR,
                        skip_group_check=True,
                    )
                if hi == 12:
                    # bias add on DVE (psum f32 + [128,1] bias -> sbuf bf16)
                    nc.vector.tensor_scalar_add(
                        qk_sb[ct][:, tsl], ps, bqk_sb[:, ct : ct + 1])

            def qkv_group_v(tb, k4, lo=0, hi=12, cell=None):
                xs = xs_tb[tb]
                tt = tb * 4 + k4
                csl = slice(k4 * 128, (k4 + 1) * 128)
                if cell is None:
                    cell = {}
                if lo == 0:
                    cell["ps"] = psp.tile([128, 512], f32, tag="qp", bufs=2,
                                          name=f"psv{tt}")
                    nc.tensor.matmul(
                        cell["ps"], ones_sb, bv_sb,
                        start=True, stop=False, skip_group_check=True,
                    )
                psv = cell["ps"]
                xh_all, xl_all = xs
                terms = [(k, kind) for k in range(EP)
                         for kind in ("hh", "hl", "lh")]
                for i in range(lo, hi):
                    k, kind = terms[i]
                    x = (xh_all if kind[1] == "h" else xl_all)[:, k, :, csl]
                    w = (wvh_t if kind[0] == "h" else wvl_t)[:, k, :, :]
                    nc.tensor.matmul(
                        psv, x, w,
                        start=False,
                        stop=(i == 11),
                        perf_mode=DR,
                        skip_group_check=True,
                    )
                if hi == 12:
                    nc.vector.tensor_copy(
                        v1_sb[:, :, tt, 0:D],
                        psv.rearrange("p (h d) -> p h d", d=D),
                    )

            def qk_chunks(tb, cts):
                steps = []
                for ct in cts:
                    cell = {}
                    for lo, hi in ((0, 4), (4, 8), (8, 12)):
                        steps.append(
                            lambda c=ct, l=lo, h=hi, ce=cell:
                            qkv_group_qk(tb, c, l, h, ce))
                return steps

            def v_chunks(tb):
                steps = []
                for g in range(4):
                    cell = {}
                    for lo, hi in ((0, 4), (4, 8), (8, 12)):
                        steps.append(
                            lambda k=g, l=lo, h=hi, ce=cell:
                            qkv_group_v(tb, k, l, h, ce))
                return steps

            def attn_block(I, nxt=(), pre=None, vpre=None):
                nj = 4 * I + 4  # causal j-tiles for this i-block
                yts = {}
                pts = {}
                ndve = {}
                dve_set = set()

                def qk_exp(pr, J):
                    r = J - 4 * I
                    ws = 128 * r if r > 0 else 0  # causal trim offset
                    qt = qk_sb[pr]
                    kt = qk_sb[4 + pr]
                    jsl = slice(J * 128, (J + 1) * 128)
                    iwl = slice(I * 512 + ws, (I + 1) * 512)
                    st = psp.tile([128, 2, 512], f32, tag="st", bufs=2,
                                  name=f"st{pr}_{I}_{J}")
                    # QK row-tile pair: head A rows 0-63, head B 64-127
                    nc.tensor.matmul(
                        st[:, 0, ws:], kt[0:64, jsl], qt[0:64, iwl],
                        tile_position=(0, 0),
                    )
                    nc.tensor.matmul(
                        st[:, 1, ws:], kt[64:128, jsl], qt[64:128, iwl],
                        tile_position=(64, 0),
                    )
                    pt = ptp.tile([128, 2, 512], bf16, tag="pt",
                                  name=f"pt{pr}_{I}_{J}")
                    # route a fraction of full tiles through the DVE exp to
                    # unload ScalarE; only in late blocks, where ScalarE is
                    # saturated and DVE is idle (early blocks are PE-paced:
                    # ScalarE starves anyway, and DVE is busy with the
                    # projection copies).  Diagonal tiles (r >= 0, need
                    # masking/trim) always use ScalarE.
                    import os
                    _m = os.environ.get('DVE_MOD', '0:0,1:5,2:4,3:4')
                    _mm = {int(a): int(b) for a, b in (p.split(':') for p in _m.split(','))}
                    mod = _mm.get(I, 0)
                    import os as _os_g
                    _hi = 4 * I - 3 if _os_g.environ.get("DVE_GUARD", "0") == "1" else 4 * I - 1
                    use_dve = (1 <= J <= _hi) and mod and \
                        (ndve["n"] % mod == 1)
                    if r < 0:
                        ndve["n"] += 1
                    if use_dve:
                        dve_set.add((pr, J))
                    if use_dve:
                        u = up.tile([128, 1024], f32, tag="u",
                                    name=f"u{pr}_{I}_{J}")
                        nc.vector._custom_dve(
                            p4_op, out=u,
                            in0=st.rearrange("p h w -> p (h w)"),
                            in1=acoef_sb, s0=CB, s1=CCo, imm2=CD)
                        nc.vector._custom_dve(
                            sq4_op, out=pt.rearrange("p h w -> p (h w)"),
                            in0=u)
                    else:
                        nc.scalar.activation(pt[:, :, ws:], st[:, :, ws:],
                                             Act.Exp, scale=0.125 / 1024.0)
                    if r >= 0:
                        # causal mask on the 128-wide diagonal band only:
                        # keep where i_band >= j (within-tile coords)
                        nc.gpsimd.affine_select(
                            out=pt[:, :, 128 * r : 128 * (r + 1)],
                            in_=pt[:, :, 128 * r : 128 * (r + 1)],
                            compare_op=is_ge,
                            fill=0.0,
                            base=0,
                            pattern=[[0, 2], [1, 128]],
                            channel_multiplier=-1,
                        )
                    pts[(pr, J)] = pt

                def pv(pr, J):
                    pt = pts.pop((pr, J))
                    ytA, ytB = yts[pr]
                    r = J - 4 * I
                    for h, yt in ((0, ytA), (1, ytB)):
                        for it in range(4):
                            if r > it:
                                continue  # i-tile fully masked for this j
                            # PSUM zeroing is bank-granular: only the FIRST
                            # region of each bank sets start=True
                            nc.tensor.matmul(
                                yt[:, it, 0 : D + 1],
                                pt[:, h, it * 128 : (it + 1) * 128],
                                v1_sb[:, 2 * pr + h, J, :],
                                start=(J == 0 and it == 0),
                                stop=(J == 4 * I + it),
                                skip_group_check=True,
                            )

                def out_stage(pr):
                    ytA, ytB = yts.pop(pr)
                    rec = op.tile([128, 2, 4], f32, tag="rec", bufs=2,
                                  name=f"rec{pr}_{I}")
                    yc = op.tile([128, 2, 4, D + 1], f32, tag="yc", bufs=2,
                                 name=f"yc{pr}_{I}")
                    ys = op.tile([128, 4, 2, D], f32, tag="ys", bufs=2,
                                 name=f"ys{pr}_{I}")
                    # copy psum->sbuf first: releases the yt banks earlier
                    nc.vector.tensor_copy(yc[:, 0, :, :], ytA[:, :, 0 : D + 1])
                    nc.vector.tensor_copy(yc[:, 1, :, :], ytB[:, :, 0 : D + 1])
                    # denominators live in column 64 of each (h, it) slot
                    nc.vector.reciprocal(rec[:, 0, :], yc[:, 0, :, D])
                    nc.vector.reciprocal(rec[:, 1, :], yc[:, 1, :, D])
                    # normalization muls on gpsimd (Pool) to unload DVE;
                    # the very last pair instead uses the by-then-idle DVE
                    # (the serialized Pool launches would pace the drain)
                    eng = nc.vector if (I == TB - 1 and pr == PAIRS - 1) \
                        else nc.gpsimd
                    for h in (0, 1):
                        for it in range(4):
                            eng.tensor_scalar_mul(
                                ys[:, it, h, :], yc[:, h, it, 0:D],
                                rec[:, h, it : it + 1])
                    # ys [i-part, it, h, d] -> y_out[I, it, p, 2pr+h, d]
                    import concourse.bass as bass

                    out_ap = bass.AP(
                        tensor=y_out.tensor,
                        offset=I * (4 * 128 * CC) + 2 * pr * D,
                        ap=[[CC, 128], [128 * CC, 4], [1, 2 * D]],
                    )
                    nc.sync.dma_start(
                        out=out_ap,
                        in_=ys.rearrange("p i h d -> p i (h d)"),
                    )

                def alloc_yt(pr):
                    # one full 2KB PSUM bank per head so the bank-granular
                    # start=True zeroing touches no other tile
                    yts[pr] = (
                        psp.tile([128, 4, 128], f32, tag="ytA", bufs=1,
                                 name=f"ytA{pr}_{I}"),
                        psp.tile([128, 4, 128], f32, tag="ytB", bufs=1,
                                 name=f"ytB{pr}_{I}"),
                    )

                ndve["n"] = I  # stagger DVE-routed tiles across blocks
                items = [(pr, J) for pr in range(PAIRS) for J in range(nj)]
                nxt = list(nxt)
                nsteps = len(nxt)
                popped = 0
                emitted = 0
                pend = []  # deferred (due, pr, J) PVs, kept due-sorted
                left = {pr: nj for pr in range(PAIRS)}

                def emit_qk(k):
                    pr, J = items[k]
                    if J == 0:
                        if pre:
                            for fn in pre.get(pr, ()):
                                fn()
                        alloc_yt(pr)
                    qk_exp(pr, J)
                    # block 0: own-block v-groups ride between the first
                    # QK emissions, clear of the exp-critical prologue
                    if vpre and pr == 0 and (J + 1) in vpre:
                        for fn in vpre.pop(J + 1):
                            fn()

                def run_pv(pr, J):
                    pv(pr, J)
                    left[pr] -= 1
                    if left[pr] == 0:
                        out_stage(pr)

                for k in range(len(items)):
                    while emitted < min(k + 4, len(items)):
                        emit_qk(emitted)
                        emitted += 1
                    # weave next t-block's QKV BEFORE this item's PV: the
                    # chunks hide inside PV's exp-wait window instead of
                    # delaying the next QK (which would starve ScalarE)
                    import os as _os
                    if _os.environ.get("SCHED_WEAVE", "pre") == "pre":
                        want = (k + 1) * nsteps // len(items)
                        while popped < want:
                            nxt[popped]()
                            popped += 1
                    pr, J = items[k]
                    # DVE-routed tiles produce pt ~2us later than ScalarE
                    # ones; defer their PV 2 items so it doesn't head-of-
                    # line-block the in-order PE queue (safe: routing is
                    # restricted to J <= 4I-3, clear of all stop flags)
                    import bisect

                    _defer = int(_os.environ.get("SCHED_DEFER", "2"))
                    due = k + _defer if (pr, J) in dve_set else k
                    bisect.insort(pend, (due, pr, J))
                    while pend and pend[0][0] <= k:
                        _, p2, J2 = pend.pop(0)
                        run_pv(p2, J2)
                    if _os.environ.get("SCHED_WEAVE", "pre") != "pre":
                        want = (k + 1) * nsteps // len(items)
                        while popped < want:
                            nxt[popped]()
                            popped += 1
                for _, p2, J2 in pend:
                    run_pv(p2, J2)
                for fn in nxt[popped:]:
                    fn()

            # schedule: per-pair staging for EVERY block.  Block I weaves
            # only what block I+1 needs at its start (v groups + pair 0's
            # q/k); pairs 1-3's q/k groups emit as `pre` inside block I+1.
            qkv_group_qk(0, 0)
            qkv_group_qk(0, 4)
            qkv_group_v(0, 0)

            def v_group_chunks(tb, g):
                cell = {}
                return [
                    lambda l=lo, h=hi, ce=cell: qkv_group_v(tb, g, l, h, ce)
                    for lo, hi in ((0, 4), (4, 8), (8, 12))
                ]

            vpre0 = {J: v_group_chunks(0, J) for J in (1, 2, 3)}
            pres = {
                0: {
                    pr: qk_chunks(0, [pr, 4 + pr])
                    for pr in range(1, PAIRS)
                }
            }
            for I in range(TB):
                nxt = []
                if I + 1 < TB:
                    load_x(I + 1)
                    nxt = v_chunks(I + 1) + qk_chunks(I + 1, [0, 4])
                    pres[I + 1] = {
                        pr: qk_chunks(I + 1, [pr, 4 + pr])
                        for pr in range(1, PAIRS)
                    }
                attn_block(I, nxt, pre=pres.get(I),
                           vpre=vpre0 if I == 0 else None)
    nc.compile()
    return nc


def get_nc():
    if "nc" not in _cache:
        _cache["nc"] = _build_nc()
    return _cache["nc"]


def _fp8_pair(a):
    import ml_dtypes

    E4 = ml_dtypes.float8_e4m3
    hi = a.astype(E4)
    lo = (a - hi.astype(np.float32)).astype(E4)
    return hi, lo


def shard_inputs(x, w_attn, b_attn):
    """Full inputs -> per-core input maps (host-side quantize/slice)."""
    x = np.asarray(x, dtype=np.float32)
    w = np.asarray(w_attn, dtype=np.float32) * WSCALE
    bb = np.asarray(b_attn, dtype=np.float32) * WSCALE
    x8h, x8l = _fp8_pair(x)  # [B, T, E] fp8, quantized once
    in_maps = []
    for core in range(N_CORES):
        b, hg = core // 2, core % 2
        r0 = hg * CC  # first q row for this head group
        # head-pair-major column packing: pr*256+[0:128]=q(pr), +[128:256]=k(pr)
        wq = w[r0 : r0 + CC, :]
        wk = w[C + r0 : C + r0 + CC, :]
        w_qk = np.concatenate(
            sum(
                (
                    [wq[pr * 128 : (pr + 1) * 128], wk[pr * 128 : (pr + 1) * 128]]
                    for pr in range(PAIRS)
                ),
                [],
            ),
            axis=0,
        ).T  # [E, 2CC]
        wqk8h, wqk8l = _fp8_pair(np.ascontiguousarray(w_qk))
        w_v = np.ascontiguousarray(w[2 * C + r0 : 2 * C + r0 + CC, :].T)
        wv8h, wv8l = _fp8_pair(w_v)
        b_qk = np.stack(
            [bb[r0 + ct * 128 : r0 + (ct + 1) * 128] for ct in range(4)]
            + [bb[C + r0 + ct * 128 : C + r0 + (ct + 1) * 128] for ct in range(4)],
            axis=1,
        ).astype(np.float32)
        b_v = bb[2 * C + r0 : 2 * C + r0 + CC].reshape(1, CC).astype(np.float32)
        in_maps.append(
            {
                "xT8h": np.ascontiguousarray(x8h[b].T),
                "xT8l": np.ascontiguousarray(x8l[b].T),
                "wqk8h": wqk8h,
                "wqk8l": wqk8l,
                "wv8h": wv8h,
                "wv8l": wv8l,
                "b_qk": np.ascontiguousarray(b_qk),
                "b_v": np.ascontiguousarray(b_v),
                "ones_d": np.ones((1, 128), dtype=np.float32),
            }
        )
    return in_maps


def run(in_maps, trace=False, **kw):
    from concourse import bass_utils

    nc = get_nc()
    return bass_utils.run_bass_kernel_spmd(
        nc, in_maps, core_ids=list(range(N_CORES)), trace=trace, **kw
    )


def gather_output(results):
    y = np.empty((B, T, E), dtype=np.float32)
    for core in range(N_CORES):
        b, hg = core // 2, core % 2
        y[b, :, hg * CC : (hg + 1) * CC] = results[core]["y_out"].reshape(T, CC)
    return y


def kernel(x, w_attn, b_attn):
    in_maps = shard_inputs(x, w_attn, b_attn)
    res = run(in_maps, trace=False)
    return gather_output(res.results)
